# revision 1
# baseline (speedup 1.0000x reference)
"""Fused decoder block (LN->QKV->cache-merge attention->proj->LN->MLP) on 8
Trainium2 NeuronCores, data-parallel over the batch (2 rows/core).

Key ideas:
- softmax is permutation-invariant over keys, so instead of scattering new
  k/v into the cache at masked slots, attend over [cache keys (masked slots
  suppressed) ++ all new keys]. The suppression is a -1e4 additive bias
  folded into the exp() activation's per-partition bias operand - zero cost.
- everything runs feature-major ([C, T] activations) so no transposes are
  needed anywhere: W as lhsT keeps activations feature-major, activations as
  lhsT produce token-major (used only for v).
- scores are computed keys-major [keys, queries]; exp'd probabilities feed
  P@V directly as the moving operand with natural-layout V as weights; an
  extra ones column on V accumulates the softmax denominator in the same
  matmuls. Per-query normalization is broadcast across partitions with a
  rank-1 PE outer product.
- LN stats (sum, sum-sq) via ones-column matmuls in fp32r; LN scale/shift
  applied via two rank<=2 PE broadcasts (g (x) rstd, g (x) -mu*rstd + b (x) 1).
- matmul dtypes: fp32r (full PE rate, ~13-bit mantissa) everywhere except
  attention qk/PV and fc2 which run bf16.
"""

import numpy as np
import ml_dtypes

B, NP, N, C, H = 16, 512, 1024, 1024, 16
HD = C // H            # 64
HID = 4 * C            # 4096
EPS = 1e-5
NCORES = 8
RPC = B // NCORES      # batch rows per core
T = NP                 # queries per row
CT = C // 128          # feature tiles
KTC = N // 128         # cache key tiles
KTN = T // 128         # new key tiles
KTA = KTC + KTN        # all key tiles
HPAIR = H // 2         # head pairs
SCALE = HD ** -0.5
MASKB = -10000.0

_state = {}


def _build_module():
    import concourse.tile as tile
    from concourse import bacc, mybir

    f32 = mybir.dt.float32
    f32r = mybir.dt.float32r
    bf16 = mybir.dt.bfloat16
    AF = mybir.ActivationFunctionType
    OP = mybir.AluOpType

    nc = bacc.Bacc("TRN2", target_bir_lowering=False, debug=False)

    xT = nc.dram_tensor("xT", [RPC, C, T], f32r, kind="ExternalInput")
    kTc = nc.dram_tensor("kTc", [RPC, H, HD, N], bf16, kind="ExternalInput")
    vc = nc.dram_tensor("vc", [RPC, H, N, HD + 1], bf16, kind="ExternalInput")
    mb = nc.dram_tensor("mb", [RPC, N], f32, kind="ExternalInput")
    wqkv = nc.dram_tensor("wqkv", [C, 3 * C], f32r, kind="ExternalInput")
    wproj = nc.dram_tensor("wproj", [C, C], f32r, kind="ExternalInput")
    wfc1 = nc.dram_tensor("wfc1", [C, HID], f32r, kind="ExternalInput")
    wfc2 = nc.dram_tensor("wfc2", [HID, C], bf16, kind="ExternalInput")
    bqkv = nc.dram_tensor("bqkv", [3 * C], f32, kind="ExternalInput")
    bproj = nc.dram_tensor("bproj", [C], f32, kind="ExternalInput")
    bfc1 = nc.dram_tensor("bfc1", [HID], f32, kind="ExternalInput")
    bfc2 = nc.dram_tensor("bfc2", [C], f32, kind="ExternalInput")
    # LN gains/biases, reshaped [CT, 128] host-side
    n1g = nc.dram_tensor("n1g", [CT, 128], f32r, kind="ExternalInput")
    n1b = nc.dram_tensor("n1b", [CT, 128], f32r, kind="ExternalInput")
    n2g = nc.dram_tensor("n2g", [CT, 128], f32r, kind="ExternalInput")
    n2b = nc.dram_tensor("n2b", [CT, 128], f32r, kind="ExternalInput")
    ones = nc.dram_tensor("ones", [128, 512], f32r, kind="ExternalInput")
    outT = nc.dram_tensor("outT", [RPC, C, T], f32, kind="ExternalOutput")

    from contextlib import ExitStack
    with nc.allow_low_precision(reason="deliberate bf16/f32r staging; accumulation stays fp32 in PSUM"), \
         tile.TileContext(nc, pool_alloc_mode="queue") as tc, ExitStack() as es:
        # ---------- constants resident for the whole kernel ----------
        consts = es.enter_context(tc.tile_pool(name="consts", bufs=1))
        ones_sb = consts.tile([128, 512], f32r)
        nc.gpsimd.dma_start(ones_sb[:], ones.ap())
        gb1 = consts.tile([2, CT, 128], f32r)
        nc.gpsimd.dma_start(gb1[0:1], n1g.ap()[None])
        nc.gpsimd.dma_start(gb1[1:2], n1b.ap()[None])
        gb2 = consts.tile([2, CT, 128], f32r)
        nc.gpsimd.dma_start(gb2[0:1], n2g.ap()[None])
        nc.gpsimd.dma_start(gb2[1:2], n2b.ap()[None])
        bqkv_sb = consts.tile([128, 16], f32)  # q,k bias columns per fchunk
        nc.gpsimd.dma_start(bqkv_sb[:], bqkv.ap()[0:2048].rearrange("(fc p) -> p fc", p=128))
        vb_sb = consts.tile([128, 2, 512], f32)  # v bias broadcast over tokens
        for ch in range(2):
            nc.gpsimd.dma_start(
                vb_sb[:, ch, :],
                bqkv.ap()[2048 + ch * 512: 2048 + (ch + 1) * 512][None].to_broadcast((128, 512)))
        bproj_sb = consts.tile([128, CT], f32)
        nc.gpsimd.dma_start(bproj_sb[:], bproj.ap().rearrange("(co p) -> p co", p=128))
        bfc1_sb = consts.tile([128, HID // 128], f32)
        nc.gpsimd.dma_start(bfc1_sb[:], bfc1.ap().rearrange("(ht p) -> p ht", p=128))
        bfc2_sb = consts.tile([128, CT], f32)
        nc.gpsimd.dma_start(bfc2_sb[:], bfc2.ap().rearrange("(co p) -> p co", p=128))
        eps_sb = consts.tile([1, 1], f32)
        nc.vector.memset(eps_sb[:], EPS)
        mb_sb = consts.tile([128, RPC, KTC], f32)
        for r in range(RPC):
            nc.gpsimd.dma_start(mb_sb[:, r, :], mb.ap()[r].rearrange("(kt p) -> p kt", p=128))

        # Pools opened/closed at phase boundaries; queue mode allows
        # non-LIFO release so each buffer spans exactly its lifetime.
        def open_pool(nm):
            cm = tc.tile_pool(name=nm, bufs=1)
            return cm, cm.__enter__()

        def close_pool(cm):
            cm.__exit__(None, None, None)

        dram_pool = es.enter_context(tc.tile_pool(name="x2d", bufs=1, space="DRAM"))
        x2ds = [[dram_pool.tile([128, T], f32, tag=f"x2d{r}_{c}", name=f"x2d{r}_{c}")
                 for c in range(CT)] for r in range(RPC)]
        cm_oT, p_oT = open_pool("p_oT")
        oTs = [[p_oT.tile([128, T], f32r, tag=f"oT{r}_{c}", name=f"oT{r}_{c}")
                for c in range(CT)] for r in range(RPC)]
        cm_h, p_h = open_pool("p_h")
        # prefetch first q/k weight chunks while LN1 runs
        cm_w0 = tc.tile_pool(name="p_w0", bufs=1)
        p_w0 = cm_w0.__enter__()
        w0_tiles = {}
        for fc in (0, 8):
            wt = p_w0.tile([128, CT, 128], f32r, tag=f"w0_{fc}", name=f"w0_{fc}")
            nc.sync.dma_start(
                wt[:], wqkv.ap()[:, fc * 128:(fc + 1) * 128]
                .rearrange("(ct p) f -> p ct f", p=128))
            w0_tiles[fc] = wt

        cm_xa, p_xa = open_pool("p_xa")
        xTs = [[p_xa.tile([128, T], f32r, tag=f"xT{r}_{ct}", name=f"xT{r}_{ct}")
                for ct in range(CT)] for r in range(RPC)]
        hTs = [[p_h.tile([128, T], f32r, tag=f"hT{r}_{ct}", name=f"hT{r}_{ct}")
                for ct in range(CT)] for r in range(RPC)]

        def layernorm(src_tiles, dst_tiles, gb, lnp, lnps, sbufs=2):
            """Feature-major layernorm src -> dst (lists of CT [128,T] tiles)."""
            s_ps = lnps.tile([1, T], f32, tag="s_ps", name="s_ps", bufs=sbufs)
            ss_ps = lnps.tile([1, T], f32, tag="ss_ps", name="ss_ps", bufs=sbufs)
            for ct in range(CT):
                nc.tensor.matmul(s_ps[:], ones_sb[:, 0:1], src_tiles[ct][:],
                                 start=(ct == 0), stop=(ct == CT - 1))
            sqs = []
            for ct in range(CT):
                sq = lnp.tile([128, T], f32r, tag="sq", name="sq", bufs=2)
                nc.vector.tensor_mul(sq[:], src_tiles[ct][:].bitcast(f32),
                                     src_tiles[ct][:].bitcast(f32))
                sqs.append(sq)
            for ct in range(CT):
                nc.tensor.matmul(ss_ps[:], ones_sb[:, 0:1], sqs[ct][:],
                                 start=(ct == 0), stop=(ct == CT - 1))
            st = lnp.tile([97, T], f32, tag="st", name="st", bufs=2)
            mean, msq, var, std = st[0:1, :], st[32:33, :], st[64:65, :], st[96:97, :]
            nc.scalar.mul(mean, s_ps[:], 1.0 / C)
            nc.vector.tensor_mul(msq, mean, mean)
            nc.vector.scalar_tensor_tensor(var, ss_ps[:], 1.0 / C, msq,
                                           OP.mult, OP.subtract)
            nc.scalar.activation(std, var, AF.Sqrt, bias=eps_sb[:])
            rstd = lnp.tile([1, T], f32r, tag="rstd", name="rstd", bufs=2)
            nc.vector.reciprocal(rstd[:], std)
            nmr = lnp.tile([2, T], f32r, tag="nmr", name="nmr", bufs=2)
            nc.vector.scalar_tensor_tensor(nmr[0:1, :], mean, -1.0,
                                           rstd[:].bitcast(f32), OP.mult, OP.mult)
            nc.sync.dma_start(nmr[1:2, :], ones.ap()[0:1, :])
            for ct in range(CT):
                a_ps = lnps.tile([128, T], f32, tag="a_ps", name="a_ps", bufs=2)
                nc.tensor.matmul(a_ps[:], gb[0:1, ct, :], rstd[:],
                                 start=True, stop=True)
                b_ps = lnps.tile([128, T], f32, tag="b_ps", name="b_ps", bufs=2)
                nc.tensor.matmul(b_ps[:], gb[:, ct, :], nmr[:],
                                 start=True, stop=True)
                t1 = lnp.tile([128, T], f32, tag="t1", name="t1", bufs=2)
                nc.vector.tensor_mul(t1[:], src_tiles[ct][:].bitcast(f32), a_ps[:])
                nc.vector.tensor_add(dst_tiles[ct][:], t1[:], b_ps[:])

        # ================= LN1 =================
        with tc.tile_pool(name="ln1", bufs=1) as lnp, \
             tc.tile_pool(name="ln1ps", bufs=1, space="PSUM") as lnps:
            for r in range(RPC):
                for ct in range(CT):
                    nc.sync.dma_start(xTs[r][ct][:], xT.ap()[r, ct * 128:(ct + 1) * 128, :])
                layernorm(xTs[r], hTs[r], gb1, lnp, lnps)

        # ========== QKV + Attention + Proj (one PSUM scope, overlapped) ==========
        close_pool(cm_xa)
        cm_qk, p_qk = open_pool("p_qk")
        cm_vn, p_vn = open_pool("p_vn")
        qTs = [[p_qk.tile([128, T], bf16, tag=f"qT{r}_{c}", name=f"qT{r}_{c}")
                for c in range(CT)] for r in range(RPC)]
        kTs = [[p_qk.tile([128, T], bf16, tag=f"kT{r}_{c}", name=f"kT{r}_{c}")
                for c in range(CT)] for r in range(RPC)]
        vns = [p_vn.tile([128, KTN, H, HD + 1], bf16, tag=f"vn{r}", name=f"vn{r}")
               for r in range(RPC)]
        def emit_qkv_chunk(fc, wqk_pool, mps):
            """One 128-col chunk of q or k for both rows."""
            if fc in w0_tiles:
                wt = w0_tiles[fc]
            else:
                wt = wqk_pool.tile([128, CT, 128], f32r, tag="wqk", name="wqk", bufs=4)
                nc.sync.dma_start(
                    wt[:], wqkv.ap()[:, fc * 128:(fc + 1) * 128]
                    .rearrange("(ct p) f -> p ct f", p=128))
            for r in range(RPC):
                ps = mps.tile([128, T], f32, tag="mm", name="mm", bufs=1)
                for ct in range(CT):
                    nc.tensor.matmul(ps[:], wt[:, ct, :], hTs[r][ct][:],
                                     start=(ct == 0), stop=(ct == CT - 1))
                dst = qTs[r][fc] if fc < 8 else kTs[r][fc - 8]
                nc.vector.tensor_scalar(
                    dst[:], ps[:], bqkv_sb[:, fc:fc + 1], None, OP.add)

        def emit_attention(hp, r, akv, asb, mps):
            kc = akv.tile([128, N], bf16, tag="kc", name="kc", bufs=3)
            nc.sync.dma_start(kc[0:64, :], kTc.ap()[r, 2 * hp])
            nc.sync.dma_start(kc[64:128, :], kTc.ap()[r, 2 * hp + 1])
            vcs = [akv.tile([128, KTC, HD + 1], bf16, tag="vcc", name="vcc", bufs=4)
                   for _ in range(2)]
            for hh in range(2):
                nc.sync.dma_start(
                    vcs[hh][:], vc.ap()[r, 2 * hp + hh]
                    .rearrange("(kt p) d -> p kt d", p=128))
            pv = [mps.tile([HD + 1, T], f32, tag="pv", name=f"pv{hh}", bufs=2)
                  for hh in range(2)]
            for kt in range(KTA):
                if kt < KTC:
                    lA = kc[0:64, kt * 128:(kt + 1) * 128]
                    lB = kc[64:128, kt * 128:(kt + 1) * 128]
                    bias = [mb_sb[:, r, kt:kt + 1], mb_sb[:, r, kt:kt + 1]]
                else:
                    ktn = kt - KTC
                    lA = kTs[r][hp][0:64, ktn * 128:(ktn + 1) * 128]
                    lB = kTs[r][hp][64:128, ktn * 128:(ktn + 1) * 128]
                    bias = [0.0, 0.0]
                s_A = mps.tile([128, T], f32, tag="sA", name="sA", bufs=2)
                s_B = mps.tile([128, T], f32, tag="sB", name="sB", bufs=2)
                nc.tensor.matmul(s_A[:], lA, qTs[r][hp][0:64, :],
                                 start=True, stop=True, tile_position=(0, 0))
                nc.tensor.matmul(s_B[:], lB, qTs[r][hp][64:128, :],
                                 start=True, stop=True, tile_position=(64, 0))
                for hh, s_ps_t in ((0, s_A), (1, s_B)):
                    p_t = asb.tile([128, T], bf16, tag="p", name="p", bufs=8)
                    nc.scalar.activation(p_t[:], s_ps_t[:], AF.Exp,
                                         bias=bias[hh], scale=SCALE)
                    lv = vcs[hh][:, kt, :] if kt < KTC else \
                        vns[r][:, kt - KTC, 2 * hp + hh, :]
                    nc.tensor.matmul(pv[hh][:], lv, p_t[:],
                                     start=(kt == 0), stop=(kt == KTA - 1))
            for hh in range(2):
                rd = asb.tile([1, T], f32r, tag="rd", name="rd", bufs=2)
                nc.vector.reciprocal(rd[:], pv[hh][HD:HD + 1, :])
                bc = mps.tile([HD, T], f32, tag="bc", name="bc", bufs=1)
                nc.tensor.matmul(bc[:], ones_sb[0:1, 0:HD], rd[:],
                                 start=True, stop=True)
                bc_sb = asb.tile([HD, T], f32, tag="bcs", name="bcs", bufs=2)
                nc.vector.tensor_copy(bc_sb[:], bc[:])
                half = oTs[r][hp][64 * hh:64 * (hh + 1), :]
                nc.vector.tensor_mul(half, pv[hh][0:HD, :], bc_sb[:])

        with tc.tile_pool(name="wqk", bufs=1) as wqk_pool, \
             tc.tile_pool(name="attn_kv", bufs=1) as akv, \
             tc.tile_pool(name="attn_sb", bufs=1) as asb, \
             tc.tile_pool(name="xb", bufs=1) as xb_pool, \
             tc.tile_pool(name="merged_ps", bufs=1, space="PSUM") as mps:
            emit_qkv_chunk(0, wqk_pool, mps)
            emit_qkv_chunk(8, wqk_pool, mps)
            for r in range(RPC):
                nc.vector.memset(vns[r][:, :, :, HD:HD + 1], 1.0)
            for ch in range(4):
                wv = wqk_pool.tile([128, CT, 256], f32r, tag="wv", name="wv", bufs=2)
                nc.sync.dma_start(
                    wv[:], wqkv.ap()[:, 2048 + ch * 256: 2048 + (ch + 1) * 256]
                    .rearrange("(ct p) f -> p ct f", p=128))
                for r in range(RPC):
                    for tt in range(KTN):
                        ps = mps.tile([128, 256], f32, tag="mm", name="mmv", bufs=1)
                        for ct in range(CT):
                            nc.tensor.matmul(
                                ps[:], hTs[r][ct][:, tt * 128:(tt + 1) * 128],
                                wv[:, ct, :], start=(ct == 0), stop=(ct == CT - 1))
                        nc.vector.tensor_add(
                            vns[r][:, tt, ch * 4:(ch + 1) * 4, 0:HD],
                            ps[:].rearrange("p (h d) -> p h d", h=4),
                            vb_sb[:, ch // 2, ch % 2 * 256:(ch % 2 + 1) * 256]
                            .rearrange("p (h d) -> p h d", h=4))
            # per head pair: q chunk, k chunk, then attention for both rows
            for hp in range(HPAIR):
                if hp > 0:
                    emit_qkv_chunk(hp, wqk_pool, mps)
                    emit_qkv_chunk(8 + hp, wqk_pool, mps)
                for r in range(RPC):
                    emit_attention(hp, r, akv, asb, mps)
        close_pool(cm_vn)
        close_pool(cm_qk)
        cm_w0.__exit__(None, None, None)
        close_pool(cm_h)

        # ================= Proj + residual =================
        cm_h2, p_h2 = open_pool("p_h2")
        h2Ts = [[p_h2.tile([128, T], f32r, tag=f"h2T{r}_{ct}", name=f"h2T{r}_{ct}")
                 for ct in range(CT)] for r in range(RPC)]
        cm_w1 = tc.tile_pool(name="w1", bufs=1)
        w1_pool = cm_w1.__enter__()
        NHT_EARLY = 4
        cm_ge, p_ge = open_pool("p_ge")
        gearly = [p_ge.tile([128, NHT_EARLY, T], bf16, tag=f"ge{r}", name=f"ge{r}")
                  for r in range(RPC)]
        cm_x2, p_x2 = open_pool("p_x2")
        x2Ts = [[p_x2.tile([128, T], f32r, tag=f"x2T{r}_{ct}", name=f"x2T{r}_{ct}")
                 for ct in range(CT)] for r in range(RPC)]
        with tc.tile_pool(name="wp", bufs=1) as wp_pool, \
             tc.tile_pool(name="xb2p", bufs=1) as xbp_pool, \
             tc.tile_pool(name="ln2", bufs=1) as lnp2, \
             tc.tile_pool(name="proj_ps", bufs=1, space="PSUM") as pps:
            for r in range(RPC):
                for co in range(CT):
                    wt = wp_pool.tile([128, CT, 128], f32r, tag="wp", name="wp", bufs=3)
                    nc.sync.dma_start(
                        wt[:], wproj.ap()[:, co * 128:(co + 1) * 128]
                        .rearrange("(ci p) f -> p ci f", p=128))
                    ps = pps.tile([128, T], f32, tag="proj", name="proj", bufs=2)
                    for ci in range(CT):
                        nc.tensor.matmul(ps[:], wt[:, ci, :], oTs[r][ci][:],
                                         start=(ci == 0), stop=(ci == CT - 1))
                    xb = xbp_pool.tile([128, T], f32, tag="xb", name="xb", bufs=4)
                    nc.gpsimd.dma_start(
                        xb[:], xT.ap()[r, co * 128:(co + 1) * 128, :].bitcast(f32))
                    nc.vector.scalar_tensor_tensor(
                        x2Ts[r][co][:], ps[:], bproj_sb[:, co:co + 1],
                        xb[:], OP.add, OP.add)
                    nc.gpsimd.dma_start(x2ds[r][co][:], x2Ts[r][co][:].bitcast(f32))
                layernorm(x2Ts[r], h2Ts[r], gb2, lnp2, pps, sbufs=1)
            # first FC1 chunks share this psum scope to overlap the LN2 tail
            for ht in range(NHT_EARLY):
                wt = w1_pool.tile([128, CT, 128], f32r, tag="w1", name="w1", bufs=4)
                nc.sync.dma_start(
                    wt[:], wfc1.ap()[:, ht * 128:(ht + 1) * 128]
                    .rearrange("(ct p) f -> p ct f", p=128))
                for r in range(RPC):
                    ps = pps.tile([128, T], f32, tag="proj", name="fc1e", bufs=2)
                    for ct in range(CT):
                        nc.tensor.matmul(ps[:], wt[:, ct, :], h2Ts[r][ct][:],
                                         start=(ct == 0), stop=(ct == CT - 1))
                    nc.scalar.activation(gearly[r][:, ht, :], ps[:], AF.Gelu,
                                         bias=bfc1_sb[:, ht:ht + 1])
        close_pool(cm_x2)

        # ================= FC1 + gelu =================
        cm_gs = []
        ghalves = [[], []]
        for r in range(RPC):
            for half in range(2):
                cm_g, p_g = open_pool(f"p_g{r}_{half}")
                cm_gs.append(cm_g)
                ghalves[r].append(p_g.tile([128, (HID // 128 - NHT_EARLY) // 2, T],
                                           bf16, tag=f"gT{r}{half}", name=f"gT{r}{half}"))

        def gslice(r, ht):
            if ht < NHT_EARLY:
                return gearly[r][:, ht, :]
            h = ht - NHT_EARLY
            nh = (HID // 128 - NHT_EARLY) // 2
            return ghalves[r][h // nh][:, h % nh, :]
        NHT = HID // 128
        with tc.tile_pool(name="w2", bufs=1) as w2_pool, \
             tc.tile_pool(name="xb2", bufs=1) as xb2_pool, \
             tc.tile_pool(name="osb", bufs=1) as osb, \
             tc.tile_pool(name="fc1_ps", bufs=1, space="PSUM") as f1ps:
            f2ps = f1ps
            for ht in range(NHT_EARLY, HID // 128):
                wt = w1_pool.tile([128, CT, 128], f32r, tag="w1", name="w1", bufs=4)
                nc.sync.dma_start(
                    wt[:], wfc1.ap()[:, ht * 128:(ht + 1) * 128]
                    .rearrange("(ct p) f -> p ct f", p=128))
                for r in range(RPC):
                    ps = f1ps.tile([128, T], f32, tag="fc1", name="fc1", bufs=4)
                    for ct in range(CT):
                        nc.tensor.matmul(ps[:], wt[:, ct, :], h2Ts[r][ct][:],
                                         start=(ct == 0), stop=(ct == CT - 1))
                    nc.scalar.activation(gslice(r, ht), ps[:], AF.Gelu,
                                         bias=bfc1_sb[:, ht:ht + 1])

            # ---- FC2 + residual -> out (same psum scope) ----
            for co in range(CT):
                whs = []
                for half in range(2):
                    wh = w2_pool.tile([128, NHT // 2, 128], bf16, tag="w2",
                                      name="w2", bufs=3)
                    nc.sync.dma_start(
                        wh[:], wfc2.ap()[half * 2048:(half + 1) * 2048,
                                         co * 128:(co + 1) * 128]
                        .rearrange("(ht p) f -> p ht f", p=128))
                    whs.append(wh)
                for r in range(RPC):
                    ps = f2ps.tile([128, T], f32, tag="fc2", name="fc2", bufs=3)
                    for ht in range(NHT):
                        nc.tensor.matmul(ps[:], whs[ht // (NHT // 2)][:, ht % (NHT // 2), :],
                                         gslice(r, ht),
                                         start=(ht == 0), stop=(ht == NHT - 1))
                    xb2 = xb2_pool.tile([128, T], f32, tag="xb2", name="xb2", bufs=3)
                    nc.gpsimd.dma_start(xb2[:], x2ds[r][co][:])
                    ot = osb.tile([128, T], f32, tag="ot", name="ot", bufs=2)
                    nc.vector.scalar_tensor_tensor(
                        ot[:], ps[:], bfc2_sb[:, co:co + 1], xb2[:], OP.add, OP.add)
                    nc.sync.dma_start(outT.ap()[r, co * 128:(co + 1) * 128, :], ot[:])
        for cm_g in reversed(cm_gs):
            close_pool(cm_g)
        close_pool(cm_ge)
        cm_w1.__exit__(None, None, None)
        close_pool(cm_h2)
        close_pool(cm_oT)

    nc.compile()
    return nc


class _Runner:
    """Hold the compiled PJRT executable (mirrors bass2jax.run_bass_via_pjrt)."""

    def __init__(self, nc, n_cores):
        import jax
        from jax.sharding import Mesh, PartitionSpec
        from jax.experimental.shard_map import shard_map
        import concourse.mybir as mybir
        from concourse.bass2jax import (
            install_neuronx_cc_hook, partition_id_tensor, _bass_exec_p)

        install_neuronx_cc_hook()
        self.jax = jax
        self.n_cores = n_cores
        partition_name = nc.partition_id_tensor.name if nc.partition_id_tensor else None
        in_names, out_names, out_avals, zero_outs = [], [], [], []
        for alloc in nc.m.functions[0].allocations:
            if not isinstance(alloc, mybir.MemoryLocationSet):
                continue
            name = alloc.memorylocations[0].name
            if alloc.kind == "ExternalInput":
                if name != partition_name:
                    in_names.append(name)
            elif alloc.kind == "ExternalOutput":
                shape = tuple(alloc.tensor_shape)
                dtype = mybir.dt.np(alloc.dtype)
                out_names.append(name)
                out_avals.append(jax.core.ShapedArray(shape, dtype))
                zero_outs.append(np.zeros(shape, dtype))
        self.in_names, self.out_names = in_names, out_names
        self.out_avals, self.zero_outs = out_avals, zero_outs
        self.n_params = len(in_names)
        all_names = in_names + out_names
        if partition_name is not None:
            all_names.append(partition_name)

        def _body(*args):
            operands = list(args)
            if partition_name is not None:
                operands.append(partition_id_tensor())
            return tuple(
                _bass_exec_p.bind(
                    *operands,
                    out_avals=tuple(out_avals),
                    in_names=tuple(all_names),
                    out_names=tuple(out_names),
                    lowering_input_output_aliases=(),
                    sim_require_finite=True,
                    sim_require_nnan=True,
                    nc=nc,
                ))

        devices = jax.devices()[:n_cores]
        assert len(devices) == n_cores, f"need {n_cores} cores, have {len(jax.devices())}"
        mesh = Mesh(np.asarray(devices), ("core",))
        n_outs = len(out_names)
        self._fn = jax.jit(
            shard_map(_body, mesh=mesh,
                      in_specs=(PartitionSpec("core"),) * (self.n_params + n_outs),
                      out_specs=(PartitionSpec("core"),) * n_outs,
                      check_rep=False),
            keep_unused=True)

    def prepare(self, in_maps):
        np_ = np
        per_core = [[np_.asarray(m[n]) for n in self.in_names] for m in in_maps]
        concat_in = [
            np_.concatenate([per_core[c][i] for c in range(self.n_cores)], axis=0)
            for i in range(self.n_params)]
        concat_zeros = [
            np_.zeros((self.n_cores * z.shape[0], *z.shape[1:]), z.dtype)
            for z in self.zero_outs]
        return self.jax.device_put(concat_in + concat_zeros)

    def run(self, prepared):
        out = self._fn(*prepared)
        self.jax.block_until_ready(out)
        return out

    def results(self, out_arrs):
        return [
            {name: np.asarray(out_arrs[i]).reshape(
                self.n_cores, *self.out_avals[i].shape)[c]
             for i, name in enumerate(self.out_names)}
            for c in range(self.n_cores)]


def _get_runner():
    if "runner" not in _state:
        nc = _build_module()
        _state["nc"] = nc
        _state["runner"] = _Runner(nc, NCORES)
    return _state["runner"]


def _prepare_in_maps(x, cache_k, cache_v, update_mask, qkv_w, qkv_b, proj_w,
                     proj_b, n1_g, n1_b, n2_g, n2_b, fc1_w, fc1_b, fc2_w, fc2_b):
    bf = ml_dtypes.bfloat16
    f32 = np.float32
    xT = np.ascontiguousarray(np.swapaxes(np.asarray(x, f32), 1, 2))          # [B,C,T]
    kTc = np.ascontiguousarray(np.swapaxes(np.asarray(cache_k, f32), 2, 3)).astype(bf)
    vc_f = np.asarray(cache_v, f32)
    vc = np.concatenate([vc_f, np.ones((*vc_f.shape[:3], 1), f32)], axis=3).astype(bf)
    mbias = np.where(np.asarray(update_mask, bool), MASKB, 0.0).astype(f32)
    shared = dict(
        wqkv=np.asarray(qkv_w, f32), wproj=np.asarray(proj_w, f32),
        wfc1=np.asarray(fc1_w, f32), wfc2=np.asarray(fc2_w, f32).astype(bf),
        bqkv=np.asarray(qkv_b, f32), bproj=np.asarray(proj_b, f32),
        bfc1=np.asarray(fc1_b, f32), bfc2=np.asarray(fc2_b, f32),
        n1g=np.asarray(n1_g, f32).reshape(CT, 128),
        n1b=np.asarray(n1_b, f32).reshape(CT, 128),
        n2g=np.asarray(n2_g, f32).reshape(CT, 128),
        n2b=np.asarray(n2_b, f32).reshape(CT, 128),
        ones=np.ones((128, 512), f32),
    )
    in_maps = []
    for c in range(NCORES):
        s = slice(c * RPC, (c + 1) * RPC)
        in_maps.append(dict(shared, xT=xT[s], kTc=kTc[s], vc=vc[s], mb=mbias[s]))
    return in_maps


def kernel(**inputs) -> np.ndarray:
    runner = _get_runner()
    in_maps = _prepare_in_maps(**inputs)
    prepared = runner.prepare(in_maps)
    out = runner.run(prepared)
    res = runner.results(out)
    full = np.empty((B, NP, C), np.float32)
    for c in range(NCORES):
        for r in range(RPC):
            full[c * RPC + r] = res[c]["outT"][r].T
    return full



# revision 10
# speedup vs baseline: 1.5230x; 1.5230x over previous
"""Fused decoder block (LN->QKV->cache-merge attention->proj->LN->MLP) on 8
Trainium2 NeuronCores, data-parallel over the batch (2 rows/core).

v2 strategy (on top of the feature-major v1):
- host-side cache compaction: update_mask marks exactly NP cache slots as
  replaced, so attention runs over [surviving 512 cache keys ++ 512 new keys]
  = 1024 keys instead of 1536, with no mask bias at all (softmax is
  permutation invariant).
- fp8e4 DoubleRow matmuls for every big GEMM (QKV, V, PV, proj, FC1, FC2):
  2 fp8 k-tiles contracted per instruction. Weights are host-quantized at
  x64 scale; activations quantized on the fly by the PSUM->SBUF copies.
- layernorm gain/bias folded into the downstream weights host-side; device
  LN is pure standardization: ones-matmul stats (sum-sq via fp8 DR on the
  squared tiles), rank-1 PE broadcasts of rstd / -mu.
- linear-layer biases enter as extra fp8 contraction rows (rank-1 DR), so
  gelu and the output copies only need scalar scale factors.
- softmax denominator from a ones column in V; its reciprocal is broadcast
  across partitions by GPSIMD partition_broadcast (Pool engine), freeing
  the PE and avoiding the two-PSUM-operand DVE restriction.
- engine balancing: exp/gelu on ACT (the roofline), quantizing copies on
  DVE, squares/adds/broadcasts on Pool.
"""

import numpy as np
import ml_dtypes

B, NP, N, C, H = 16, 512, 1024, 1024, 16
HD = C // H            # 64
HID = 4 * C            # 4096
EPS = 1e-5
NCORES = 8
RPC = B // NCORES      # batch rows per core
T = NP                 # queries per row
CT = C // 128          # feature tiles
NKC = N - NP           # surviving cache keys after compaction (512)
KTC = NKC // 128       # cache key tiles (4)
KTN = T // 128         # new key tiles (4)
KT = KTC + KTN         # total key tiles (8)
HPAIR = H // 2         # head pairs
NHT = HID // 128       # fc1 output chunks (32)
SCALE = HD ** -0.5
WS = 64.0              # weight quantization scale
OS = 16.0              # v / attention-output scale
VP = 80                # padded v row: 64 d + 1 ones + 15 pad (16B alignment)

_state = {}
fp8np = ml_dtypes.float8_e4m3


def _build_module():
    import concourse.tile as tile
    from concourse import bacc, mybir

    f32 = mybir.dt.float32
    f32r = mybir.dt.float32r
    fp8 = mybir.dt.float8e4
    AF = mybir.ActivationFunctionType
    OP = mybir.AluOpType
    DRm = mybir.MatmulPerfMode.DoubleRow

    nc = bacc.Bacc("TRN2", target_bir_lowering=False, debug=False)

    xT = nc.dram_tensor("xT", [RPC, 128, CT, T], f32r, kind="ExternalInput")
    kcC = nc.dram_tensor("kcC", [RPC, 128, HPAIR, NKC], fp8, kind="ExternalInput")
    vcC = nc.dram_tensor("vcC", [RPC, 128, H * KTC * VP], fp8, kind="ExternalInput")
    wqk = nc.dram_tensor("wqk", [16, 128, CT, 128], fp8, kind="ExternalInput")
    wv = nc.dram_tensor("wv", [4, 128, CT, 256], fp8, kind="ExternalInput")
    wp = nc.dram_tensor("wp", [CT, 128, CT, 128], fp8, kind="ExternalInput")
    w1 = nc.dram_tensor("w1", [NHT, 128, 2, CT, 128], fp8, kind="ExternalInput")
    w2 = nc.dram_tensor("w2", [CT, 128, 2, NHT, 128], fp8, kind="ExternalInput")
    wxp = nc.dram_tensor("wxp", [1, CT, 2, 128], fp8, kind="ExternalInput")
    wx1 = nc.dram_tensor("wx1", [1, NHT, 2, 128], fp8, kind="ExternalInput")
    wx2 = nc.dram_tensor("wx2", [1, CT, 2, 128], fp8, kind="ExternalInput")
    bqk = nc.dram_tensor("bqk", [128, 16], f32, kind="ExternalInput")
    vbias = nc.dram_tensor("vbias", [C], f32, kind="ExternalInput")
    ones = nc.dram_tensor("ones", [128, 512], f32r, kind="ExternalInput")
    outT = nc.dram_tensor("outT", [RPC, 128, CT, T], f32, kind="ExternalOutput")

    from contextlib import ExitStack
    with nc.allow_low_precision(reason="deliberate fp8/f32r staging; PSUM accumulation fp32"), \
         tile.TileContext(nc, pool_alloc_mode="queue") as tc, ExitStack() as es:
        # ---------- constants ----------
        consts = es.enter_context(tc.tile_pool(name="consts", bufs=1))
        ones_sb = consts.tile([128, 512], f32r)
        nc.sync.dma_start(ones_sb[:], ones.ap())
        ones8 = consts.tile([128, 2, 16], fp8)
        nc.vector.memset(ones8[:], 1.0)
        bqk_sb = consts.tile([128, 16], f32)
        nc.sync.dma_start(bqk_sb[:], bqk.ap())
        vb_sb = consts.tile([128, C], f32)
        nc.sync.dma_start(vb_sb[:], vbias.ap()[None].to_broadcast((128, C)))
        wxp_sb = consts.tile([1, CT, 2, 128], fp8)
        nc.sync.dma_start(wxp_sb[:], wxp.ap())
        wx1_sb = consts.tile([1, NHT, 2, 128], fp8)
        nc.sync.dma_start(wx1_sb[:], wx1.ap())
        wx2_sb = consts.tile([1, CT, 2, 128], fp8)
        nc.sync.dma_start(wx2_sb[:], wx2.ap())
        xtr_p = consts.tile([1, 2, 512], fp8)   # moving rows for proj extra
        nc.vector.memset(xtr_p[0:1, 0, :], OS)
        nc.vector.memset(xtr_p[0:1, 1, :], 0.0)
        xtr_1 = consts.tile([1, 2, 512], fp8)   # moving rows for fc1/fc2 extra
        nc.vector.memset(xtr_1[0:1, 0, :], 1.0)
        nc.vector.memset(xtr_1[0:1, 1, :], 0.0)
        eps_sb = consts.tile([1, 1], f32)
        nc.vector.memset(eps_sb[:], EPS)

        def open_pool(nm, space=None):
            kw = dict(space=space) if space else {}
            cm = tc.tile_pool(name=nm, bufs=1, **kw)
            return cm, cm.__enter__()

        def close_pool(cm):
            cm.__exit__(None, None, None)

        # ---------- pools ordered for ring-stack discipline ----------
        # xs and oT open first and close last; x2/g/h2 open after the
        # attention-phase pools close, so attention SBUF pressure stays low.
        cm_xs, p_xs = open_pool("p_xs")     # x residual
        xs = [p_xs.tile([128, CT, T], f32r, tag=f"xs{r}", name=f"xs{r}")
              for r in range(RPC)]
        cm_oT, p_oT = open_pool("p_oT")     # attention output
        oT = [p_oT.tile([128, CT, T], fp8, tag=f"oT{r}", name=f"oT{r}")
              for r in range(RPC)]
        cm_h1, p_h1 = open_pool("p_h1")     # LN1 out: LN1 -> attention end
        h1 = [p_h1.tile([128, CT, T], fp8, tag=f"h1{r}", name=f"h1{r}")
              for r in range(RPC)]

        # ================= layernorm (standardize only) =================
        def layernorm(src, dst, lnp, lnps, dst_lo=None):
            """src [128,CT,T] f32r -> dst fp8 standardized (+ optional lo residual)."""
            s_ps = lnps.tile([1, T], f32, tag="s_ps", name="s_ps", bufs=1)
            for ct in range(CT):
                nc.tensor.matmul(s_ps[:], ones_sb[:, 0:1], src[:, ct, :],
                                 start=(ct == 0), stop=(ct == CT - 1))
            ss_ps = lnps.tile([1, T], f32, tag="ss_ps", name="ss_ps", bufs=1)
            for c in range(CT // 2):
                sqs = lnp.tile([128, 2, T], fp8, tag="sqs", name="sqs", bufs=2)
                for j in range(2):
                    eng = nc.gpsimd if j else nc.vector
                    eng.tensor_mul(sqs[:, j, :], src[:, 2 * c + j, :].bitcast(f32),
                                   src[:, 2 * c + j, :].bitcast(f32))
                nc.tensor.matmul(ss_ps[:], ones8[:, :, 0:1], sqs[:],
                                 start=(c == 0), stop=(c == CT // 2 - 1),
                                 perf_mode=DRm)
            st = lnp.tile([33, T], f32, tag="st", name="st", bufs=1)
            msq, var = st[0:1, :], st[32:33, :]
            negmu = lnp.tile([1, T], f32r, tag="negmu", name="negmu", bufs=1)
            nc.vector.tensor_scalar(negmu[:], s_ps[:], -1.0 / C, None, OP.mult)
            nc.vector.tensor_mul(msq, negmu[:].bitcast(f32), negmu[:].bitcast(f32))
            nc.vector.scalar_tensor_tensor(var, ss_ps[:], 1.0 / C, msq,
                                           OP.mult, OP.subtract)
            stdv = lnp.tile([1, T], f32, tag="stdv", name="stdv", bufs=1)
            nc.scalar.activation(stdv[:], var, AF.Sqrt, bias=eps_sb[:])
            rstd = lnp.tile([1, T], f32r, tag="rstd", name="rstd", bufs=1)
            nc.vector.reciprocal(rstd[:], stdv[:])
            am_ps = lnps.tile([128, 2, T], f32, tag="am", name="am", bufs=1)
            nc.tensor.matmul(am_ps[:, 0, :], ones_sb[0:1, 0:128], rstd[:],
                             start=True, stop=True)
            nc.tensor.matmul(am_ps[:, 1, :], ones_sb[0:1, 0:128],
                             negmu[:], start=True, stop=True)
            am_sb = lnp.tile([128, 2, T], f32, tag="amsb", name="amsb", bufs=1)
            nc.vector.tensor_copy(am_sb[:, 0, :], am_ps[:, 0, :])
            nc.vector.tensor_copy(am_sb[:, 1, :], am_ps[:, 1, :])
            for ct in range(CT):
                tmp = lnp.tile([128, T], f32, tag="tmp", name="tmp", bufs=2)
                nc.gpsimd.tensor_add(tmp[:], src[:, ct, :].bitcast(f32),
                                     am_sb[:, 1, :])
                if dst_lo is None:
                    nc.vector.tensor_mul(dst[:, ct, :], tmp[:], am_sb[:, 0, :])
                else:
                    hf = lnp.tile([128, T], f32, tag="hf", name="hf", bufs=2)
                    nc.vector.tensor_mul(hf[:], tmp[:], am_sb[:, 0, :])
                    nc.gpsimd.tensor_copy(dst[:, ct, :], hf[:])
                    nc.vector.scalar_tensor_tensor(
                        dst_lo[:, ct, :], dst[:, ct, :], -1.0, hf[:],
                        OP.mult, OP.add)

        cm_ln1, p_ln1 = open_pool("ln1")
        cm_lnps, p_lnps = open_pool("ln1ps", space="PSUM")
        for r in range(RPC):
            nc.sync.dma_start(xs[r][:], xT.ap()[r])
            layernorm(xs[r], h1[r], p_ln1, p_lnps)
        close_pool(cm_lnps)
        close_pool(cm_ln1)

        # ================= QKV + attention =================
        cm_qk, p_qk = open_pool("p_qk")
        q_sb = [p_qk.tile([128, CT, T], fp8, tag=f"q{r}", name=f"q{r}")
                for r in range(RPC)]
        k_sb = [p_qk.tile([128, CT, T], fp8, tag=f"k{r}", name=f"k{r}")
                for r in range(RPC)]
        cm_kv, p_kv = open_pool("p_kv")
        kc_sb = [p_kv.tile([128, HPAIR, NKC], fp8, tag=f"kc{r}", name=f"kc{r}")
                 for r in range(RPC)]
        vkv = [p_kv.tile([128, 2, H, KTC, VP], fp8, tag=f"vkv{r}", name=f"vkv{r}")
               for r in range(RPC)]
        for r in range(RPC):
            nc.sync.dma_start(kc_sb[r][:], kcC.ap()[r])
            nc.sync.dma_start(
                vkv[r][:, 0, :, :, :].rearrange("p h k d -> p (h k d)"),
                vcC.ap()[r])
            # new-v half: ones column + zero pad
            nc.gpsimd.memset(vkv[r][:, 1, :, :, HD:HD + 1], 1.0)
            nc.gpsimd.memset(vkv[r][:, 1, :, :, HD + 1:VP], 0.0)
        cm_wq, wpool = open_pool("p_w")
        cm_att, apool = open_pool("p_att")
        cm_mmps, mmps = open_pool("p_mmps", space="PSUM")
        cm_scps, scps = open_pool("p_scps", space="PSUM")

        def emit_qk_chunk(i):
            wt = wpool.tile([128, CT, 128], fp8, tag="wqk", name="wqk", bufs=3)
            nc.sync.dma_start(wt[:], wqk.ap()[i])
            for r in range(RPC):
                ps = mmps.tile([128, T], f32, tag="mm", name="mm", bufs=2)
                for c in range(CT // 2):
                    nc.tensor.matmul(ps[:], wt[:, 2 * c:2 * c + 2, :],
                                     h1[r][:, 2 * c:2 * c + 2, :],
                                     start=(c == 0), stop=(c == CT // 2 - 1),
                                     perf_mode=DRm)
                dst = q_sb[r][:, i, :] if i < 8 else k_sb[r][:, i - 8, :]
                nc.vector.tensor_scalar(dst, ps[:], 1.0 / WS,
                                        bqk_sb[:, i:i + 1], OP.mult, OP.add)

        def emit_v_chunk(ch):
            wvt = wpool.tile([128, CT, 256], fp8, tag="wv", name="wv", bufs=2)
            nc.sync.dma_start(wvt[:], wv.ap()[ch])
            for r in range(RPC):
                for tt in range(KTN):
                    psf = mmps.tile([128, T], f32, tag="mm", name="mm", bufs=2)
                    ps = psf[:, 0:256]
                    for c in range(CT // 2):
                        nc.tensor.matmul(
                            ps, h1[r][:, 2 * c:2 * c + 2, tt * 128:(tt + 1) * 128],
                            wvt[:, 2 * c:2 * c + 2, :],
                            start=(c == 0), stop=(c == CT // 2 - 1), perf_mode=DRm)
                    nc.vector.scalar_tensor_tensor(
                        vkv[r][:, 1, 4 * ch:4 * ch + 4, tt, 0:HD],
                        ps.rearrange("p (h d) -> p h d", h=4), OS / WS,
                        vb_sb[:, ch * 256:(ch + 1) * 256]
                        .rearrange("p (h d) -> p h d", h=4),
                        OP.mult, OP.add)

        def emit_attention(hp, r):
            for hh in range(2):
                h = 2 * hp + hh
                pt = apool.tile([128, KT, T], fp8, tag="pt", name="pt", bufs=2)
                pv = scps.tile([128, T], f32, tag="pv", name="pv", bufs=2)
                for g in range(KT // 2):
                    sc = scps.tile([128, 2, T], f32, tag="sc", name="sc", bufs=2)
                    for j in range(2):
                        kt = 2 * g + j
                        if kt < KTC:
                            lhs = kc_sb[r][64 * hh:64 * hh + 64, hp,
                                           kt * 128:(kt + 1) * 128]
                        else:
                            lhs = k_sb[r][64 * hh:64 * hh + 64, hp,
                                          (kt - KTC) * 128:(kt - KTC + 1) * 128]
                        nc.tensor.matmul(sc[:, j, :], lhs,
                                         q_sb[r][64 * hh:64 * hh + 64, hp, :],
                                         start=True, stop=True,
                                         tile_position=(64 * hh, 0))
                    nc.scalar.activation(
                        pt[:, 2 * g:2 * g + 2, :].rearrange("p a b -> p (a b)"),
                        sc[:].rearrange("p a b -> p (a b)"), AF.Exp, scale=SCALE)
                    half = 0 if g < KTC // 2 else 1
                    koff = 0 if g < KTC // 2 else KTC
                    nc.tensor.matmul(
                        pv[0:VP, :],
                        vkv[r][:, half, h, 2 * g - koff:2 * g - koff + 2, :],
                        pt[:, 2 * g:2 * g + 2, :],
                        start=(g == 0), stop=(g == KT // 2 - 1), perf_mode=DRm)
                rd = apool.tile([1, T], f32, tag="rd", name="rd", bufs=2)
                nc.vector.reciprocal(rd[:], pv[HD:HD + 1, :])
                bcs = apool.tile([HD, T], f32, tag="bcs", name="bcs", bufs=2)
                nc.gpsimd.partition_broadcast(bcs[:], rd[:])
                nc.vector.tensor_mul(oT[r][64 * hh:64 * hh + 64, hp, :],
                                     pv[0:HD, :], bcs[:])

        emit_qk_chunk(0)
        emit_qk_chunk(8)
        emit_v_chunk(0)
        for hp in range(HPAIR):
            if hp > 0:
                emit_qk_chunk(hp)
                emit_qk_chunk(8 + hp)
            if hp in (1, 3, 5):
                emit_v_chunk((hp + 1) // 2)
            for r in range(RPC):
                emit_attention(hp, r)
        close_pool(cm_scps)
        close_pool(cm_mmps)
        close_pool(cm_att)
        close_pool(cm_wq)
        close_pool(cm_kv)
        close_pool(cm_qk)
        close_pool(cm_h1)

        # ================= proj + residual + LN2 =================
        cm_x2, p_x2 = open_pool("p_x2")
        cm_g, p_g = open_pool("p_g")
        cm_h2, p_h2 = open_pool("p_h2")
        x2s = [p_x2.tile([128, CT, T], f32r, tag=f"x2{r}", name=f"x2{r}")
               for r in range(RPC)]
        h2 = [p_h2.tile([128, 2, CT, T], fp8, tag=f"h2{r}", name=f"h2{r}")
              for r in range(RPC)]
        with tc.tile_pool(name="wp", bufs=1) as wp_pool, \
             tc.tile_pool(name="ln2", bufs=1) as lnp2, \
             tc.tile_pool(name="proj_ps", bufs=1, space="PSUM") as pps:
            for co in range(CT):
                wt = wp_pool.tile([128, CT, 128], fp8, tag="wp", name="wp", bufs=3)
                nc.sync.dma_start(wt[:], wp.ap()[co])
                for r in range(RPC):
                    ps = pps.tile([128, T], f32, tag="proj", name="proj", bufs=2)
                    for c in range(CT // 2):
                        nc.tensor.matmul(ps[:], wt[:, 2 * c:2 * c + 2, :],
                                         oT[r][:, 2 * c:2 * c + 2, :],
                                         start=(c == 0), stop=False, perf_mode=DRm)
                    nc.tensor.matmul(ps[:], wxp_sb[0:1, co, :, :], xtr_p[0:1, :, :],
                                     start=False, stop=True, perf_mode=DRm)
                    nc.vector.scalar_tensor_tensor(
                        x2s[r][:, co, :], ps[:], 1.0 / (OS * WS),
                        xs[r][:, co, :].bitcast(f32), OP.mult, OP.add)
            for r in range(RPC):
                layernorm(x2s[r], h2[r][:, 0], lnp2, pps, dst_lo=h2[r][:, 1])

        # ================= FC1 + gelu =================
        gs = [p_g.tile([128, NHT, T], fp8, tag=f"g{r}", name=f"g{r}")
              for r in range(RPC)]
        cm_w1, w1_pool = open_pool("w1p")
        with tc.tile_pool(name="fc1_ps", bufs=1, space="PSUM") as f1ps:
            for htp in range(NHT // 2):
                gps = [None, None]
                for j in range(2):
                    ht = 2 * htp + j
                    wt = w1_pool.tile([128, 2, CT, 128], fp8, tag="w1", name="w1", bufs=4)
                    nc.sync.dma_start(wt[:], w1.ap()[ht])
                    for r in range(RPC):
                        if j == 0:
                            gps[r] = f1ps.tile([128, 2, T], f32, tag="f1",
                                               name="f1", bufs=3)
                        ps = gps[r]
                        # pass 0: W_hi @ h_hi ; pass 1: W_hi @ h_lo ; pass 2: W_lo @ h_hi
                        for pw, ph in ((0, 0), (0, 1), (1, 0)):
                            for c in range(CT // 2):
                                nc.tensor.matmul(ps[:, j, :],
                                                 wt[:, pw, 2 * c:2 * c + 2, :],
                                                 h2[r][:, ph, 2 * c:2 * c + 2, :],
                                                 start=(pw == 0 and ph == 0 and c == 0),
                                                 stop=False, perf_mode=DRm)
                        nc.tensor.matmul(ps[:, j, :], wx1_sb[0:1, ht, :, :],
                                         xtr_1[0:1, :, :],
                                         start=False, stop=True, perf_mode=DRm)
                for r in range(RPC):
                    nc.scalar.activation(
                        gs[r][:, 2 * htp:2 * htp + 2, :].rearrange("p a b -> p (a b)"),
                        gps[r][:].rearrange("p a b -> p (a b)"),
                        AF.Gelu, scale=1.0 / WS)
        close_pool(cm_w1)
        close_pool(cm_h2)

        # ================= FC2 + residual -> out =================
        with tc.tile_pool(name="w2p", bufs=1) as w2_pool, \
             tc.tile_pool(name="osb", bufs=1) as osb, \
             tc.tile_pool(name="fc2_ps", bufs=1, space="PSUM") as f2ps:
            for co in range(CT):
                wt = w2_pool.tile([128, 2, NHT, 128], fp8, tag="w2", name="w2", bufs=2)
                nc.sync.dma_start(wt[:], w2.ap()[co])
                for r in range(RPC):
                    ps = f2ps.tile([128, T], f32, tag="fc2", name="fc2", bufs=3)
                    for pw in range(2):
                        for tpair in range(NHT // 2):
                            nc.tensor.matmul(ps[:],
                                             wt[:, pw, 2 * tpair:2 * tpair + 2, :],
                                             gs[r][:, 2 * tpair:2 * tpair + 2, :],
                                             start=(pw == 0 and tpair == 0),
                                             stop=False, perf_mode=DRm)
                    nc.tensor.matmul(ps[:], wx2_sb[0:1, co, :, :], xtr_1[0:1, :, :],
                                     start=False, stop=True, perf_mode=DRm)
                    ot = osb.tile([128, T], f32, tag="ot", name="ot", bufs=3)
                    nc.vector.scalar_tensor_tensor(
                        ot[:], ps[:], 1.0 / WS, x2s[r][:, co, :].bitcast(f32),
                        OP.mult, OP.add)
                    nc.sync.dma_start(outT.ap()[r, :, co, :], ot[:])
        close_pool(cm_g)
        close_pool(cm_x2)
        close_pool(cm_oT)
        close_pool(cm_xs)

    nc.compile()
    return nc


class _Runner:
    """Hold the compiled PJRT executable (mirrors bass2jax.run_bass_via_pjrt)."""

    def __init__(self, nc, n_cores):
        import jax
        from jax.sharding import Mesh, PartitionSpec
        from jax.experimental.shard_map import shard_map
        import concourse.mybir as mybir
        from concourse.bass2jax import (
            install_neuronx_cc_hook, partition_id_tensor, _bass_exec_p)

        install_neuronx_cc_hook()
        self.jax = jax
        self.n_cores = n_cores
        partition_name = nc.partition_id_tensor.name if nc.partition_id_tensor else None
        in_names, out_names, out_avals, zero_outs = [], [], [], []
        for alloc in nc.m.functions[0].allocations:
            if not isinstance(alloc, mybir.MemoryLocationSet):
                continue
            name = alloc.memorylocations[0].name
            if alloc.kind == "ExternalInput":
                if name != partition_name:
                    in_names.append(name)
            elif alloc.kind == "ExternalOutput":
                shape = tuple(alloc.tensor_shape)
                dtype = mybir.dt.np(alloc.dtype)
                out_names.append(name)
                out_avals.append(jax.core.ShapedArray(shape, dtype))
                zero_outs.append(np.zeros(shape, dtype))
        self.in_names, self.out_names = in_names, out_names
        self.out_avals, self.zero_outs = out_avals, zero_outs
        self.n_params = len(in_names)
        all_names = in_names + out_names
        if partition_name is not None:
            all_names.append(partition_name)

        def _body(*args):
            operands = list(args)
            if partition_name is not None:
                operands.append(partition_id_tensor())
            return tuple(
                _bass_exec_p.bind(
                    *operands,
                    out_avals=tuple(out_avals),
                    in_names=tuple(all_names),
                    out_names=tuple(out_names),
                    lowering_input_output_aliases=(),
                    sim_require_finite=True,
                    sim_require_nnan=True,
                    nc=nc,
                ))

        devices = jax.devices()[:n_cores]
        assert len(devices) == n_cores, f"need {n_cores} cores, have {len(jax.devices())}"
        mesh = Mesh(np.asarray(devices), ("core",))
        n_outs = len(out_names)
        self._fn = jax.jit(
            shard_map(_body, mesh=mesh,
                      in_specs=(PartitionSpec("core"),) * (self.n_params + n_outs),
                      out_specs=(PartitionSpec("core"),) * n_outs,
                      check_rep=False),
            keep_unused=True)

    def prepare(self, in_maps):
        np_ = np
        per_core = [[np_.asarray(m[n]) for n in self.in_names] for m in in_maps]
        concat_in = [
            np_.concatenate([per_core[c][i] for c in range(self.n_cores)], axis=0)
            for i in range(self.n_params)]
        concat_zeros = [
            np_.zeros((self.n_cores * z.shape[0], *z.shape[1:]), z.dtype)
            for z in self.zero_outs]
        return self.jax.device_put(concat_in + concat_zeros)

    def run(self, prepared):
        out = self._fn(*prepared)
        self.jax.block_until_ready(out)
        return out

    def results(self, out_arrs):
        return [
            {name: np.asarray(out_arrs[i]).reshape(
                self.n_cores, *self.out_avals[i].shape)[c]
             for i, name in enumerate(self.out_names)}
            for c in range(self.n_cores)]


def _get_runner():
    if "runner" not in _state:
        nc = _build_module()
        _state["nc"] = nc
        _state["runner"] = _Runner(nc, NCORES)
    return _state["runner"]


def _prepare_in_maps(x, cache_k, cache_v, update_mask, qkv_w, qkv_b, proj_w,
                     proj_b, n1_g, n1_b, n2_g, n2_b, fc1_w, fc1_b, fc2_w, fc2_b):
    f32 = np.float32
    x = np.asarray(x, f32)
    cache_k = np.asarray(cache_k, f32)
    cache_v = np.asarray(cache_v, f32)
    update_mask = np.asarray(update_mask, bool)
    qkv_w = np.asarray(qkv_w, f32)
    qkv_b = np.asarray(qkv_b, f32)
    proj_w = np.asarray(proj_w, f32)
    proj_b = np.asarray(proj_b, f32)
    n1_g = np.asarray(n1_g, f32)
    n1_b = np.asarray(n1_b, f32)
    n2_g = np.asarray(n2_g, f32)
    n2_b = np.asarray(n2_b, f32)
    fc1_w = np.asarray(fc1_w, f32)
    fc1_b = np.asarray(fc1_b, f32)
    fc2_w = np.asarray(fc2_w, f32)
    fc2_b = np.asarray(fc2_b, f32)

    # x feature-major [B, 128, CT, T]
    xT = np.ascontiguousarray(
        x.transpose(0, 2, 1).reshape(B, CT, 128, T).transpose(0, 2, 1, 3))

    # cache compaction: keep slots where update_mask is False
    kcC = np.empty((B, 128, HPAIR, NKC), fp8np)
    vcC = np.empty((B, 128, H, KTC, VP), fp8np)
    for b in range(B):
        keep = ~update_mask[b]
        kc = cache_k[b][:, keep, :]          # [H, NKC, HD]
        vc = cache_v[b][:, keep, :]
        # kcC[b, parity*64+hd, hp, key] = kc[2hp+parity, key, hd]
        kcC[b] = (kc.transpose(0, 2, 1).reshape(HPAIR, 2, HD, NKC)
                  .transpose(1, 2, 0, 3).reshape(128, HPAIR, NKC).astype(fp8np))
        # vcC[b, p, h, kt, d] = OS*vc[h, kt*128+p, d]; ones col at d=64
        vv = (vc.transpose(1, 0, 2).reshape(KTC, 128, H, HD)
              .transpose(1, 2, 0, 3))        # [128, H, KTC, HD]
        pad = np.zeros((128, H, KTC, VP - HD), f32)
        pad[:, :, :, 0] = 1.0
        vcC[b] = np.concatenate([vv * OS, pad], axis=3).astype(fp8np)

    def wtile(w, nf, cols):
        # w [C_in, nf*cols] -> [nf, 128, C_in//128, cols]
        ci = w.shape[0]
        return np.ascontiguousarray(
            (WS * w).reshape(ci // 128, 128, nf, cols)
            .transpose(2, 1, 0, 3)).astype(fp8np)

    def wtile_hl(w, nf, cols):
        # hi/lo residual pair -> [nf, 128, 2, C_in//128, cols]
        ws = WS * w
        hi = ws.astype(fp8np)
        lo = (ws - hi.astype(np.float32)).astype(fp8np)
        ci = w.shape[0]

        def t(a):
            return (a.reshape(ci // 128, 128, nf, cols).transpose(2, 1, 0, 3))
        return np.ascontiguousarray(
            np.stack([t(hi), t(lo)], axis=2)).astype(fp8np)

    wqkv_eff = n1_g[:, None] * qkv_w
    bias_qkv = n1_b @ qkv_w + qkv_b
    wqk_t = wtile(wqkv_eff[:, :2048], 16, 128)
    wv_t = wtile(wqkv_eff[:, 2048:], 4, 256)
    bqk_t = np.ascontiguousarray(bias_qkv[:2048].reshape(16, 128).T).astype(f32)
    vbias_t = (OS * bias_qkv[2048:]).astype(f32)

    wp_t = wtile(proj_w, CT, 128)
    wxp_t = np.zeros((1, CT, 2, 128), fp8np)
    wxp_t[0, :, 0, :] = (WS * proj_b).reshape(CT, 128).astype(fp8np)

    w1_eff = n2_g[:, None] * fc1_w
    bias_fc1 = n2_b @ fc1_w + fc1_b
    w1_t = wtile_hl(w1_eff, NHT, 128)
    wx1_t = np.zeros((1, NHT, 2, 128), fp8np)
    wx1_t[0, :, 0, :] = (WS * bias_fc1).reshape(NHT, 128).astype(fp8np)

    w2_t = wtile_hl(fc2_w, CT, 128)
    wx2_t = np.zeros((1, CT, 2, 128), fp8np)
    wx2_t[0, :, 0, :] = (WS * fc2_b).reshape(CT, 128).astype(fp8np)

    shared = dict(
        wqk=wqk_t, wv=wv_t, wp=wp_t, w1=w1_t, w2=w2_t,
        wxp=wxp_t, wx1=wx1_t, wx2=wx2_t,
        bqk=bqk_t, vbias=vbias_t,
        ones=np.ones((128, 512), f32),
    )
    in_maps = []
    for c in range(NCORES):
        s = slice(c * RPC, (c + 1) * RPC)
        in_maps.append(dict(
            shared, xT=xT[s], kcC=kcC[s],
            vcC=vcC[s].reshape(RPC, 128, H * KTC * VP)))
    return in_maps


def kernel(**inputs) -> np.ndarray:
    runner = _get_runner()
    in_maps = _prepare_in_maps(**inputs)
    prepared = runner.prepare(in_maps)
    out = runner.run(prepared)
    res = runner.results(out)
    full = np.empty((B, NP, C), np.float32)
    for c in range(NCORES):
        for r in range(RPC):
            # outT[r] [128, CT, T] -> [T, C] with c = ct*128 + p
            full[c * RPC + r] = res[c]["outT"][r].transpose(2, 1, 0).reshape(T, C)
    return full


# revision 26
# speedup vs baseline: 1.6408x; 1.0773x over previous
"""Fused decoder block (LN->QKV->cache-merge attention->proj->LN->MLP) on 8
Trainium2 NeuronCores, data-parallel over the batch (2 rows/core).

v3: row-pipelined schedule. The two batch rows per core are independent, so
the ACT-bound attention of row 1 overlaps the PE-bound proj/LN2/FC1 of row 0.

Key ideas (cumulative):
- host-side cache compaction via update_mask: attention over [512 surviving
  cache keys ++ 512 new keys] = 1024 keys, no mask bias (softmax is
  permutation invariant).
- fp8e4 DoubleRow matmuls everywhere (QKV, scores, PV, proj, FC1, FC2);
  weights host-quantized at x64 scale; hi+lo residual passes for W1/W2/h2
  keep the MLP path accurate (rel err ~1.3e-2 < 2e-2).
- q/k swizzled into [32, 2, head, T] pair layout by SBUF->SBUF DMAs so the
  64-dim head contraction runs as DoubleRow (half cost).
- LN standardization only (gains/biases folded into weights host-side);
  sum-sq stats via fp8 DR on squared tiles; rstd/-mu broadcast across
  partitions by GPSIMD partition_broadcast (no PE, no PSUM).
- linear biases as extra fp8 contraction rows; softmax denominator from a
  ones column in V, reciprocal broadcast on Pool.
- x2 residual kept in bf16 to fit SBUF; PSUM pools sized to exactly 8 banks
  per phase.
"""

import numpy as np
import ml_dtypes

B, NP, N, C, H = 16, 512, 1024, 1024, 16
HD = C // H            # 64
HID = 4 * C            # 4096
EPS = 1e-5
NCORES = 8
RPC = B // NCORES      # batch rows per core
T = NP                 # queries per row
CT = C // 128          # feature tiles
NKC = N - NP           # surviving cache keys (512)
KTC = NKC // 128       # cache key tiles (4)
KTN = T // 128         # new key tiles (4)
KT = KTC + KTN         # total key tiles (8)
HPAIR = H // 2
NHT = HID // 128       # fc1 output chunks (32)
SCALE = HD ** -0.5
WS = 64.0              # weight quantization scale
OS = 16.0              # v / attention-output scale
VP = 80                # padded v row (64 d + 1 ones + 15 pad)

_state = {}
fp8np = ml_dtypes.float8_e4m3


def _build_module(with_bias=True):
    import concourse.tile as tile
    from concourse import bacc, mybir

    f32 = mybir.dt.float32
    f32r = mybir.dt.float32r
    bf16 = mybir.dt.bfloat16
    fp8 = mybir.dt.float8e4
    AF = mybir.ActivationFunctionType
    OP = mybir.AluOpType
    DRm = mybir.MatmulPerfMode.DoubleRow

    nc = bacc.Bacc("TRN2", target_bir_lowering=False, debug=False)

    xT = nc.dram_tensor("xT", [RPC, 128, CT, T], f32r, kind="ExternalInput")
    kcD = nc.dram_tensor("kcD", [RPC, 128, 2, 4, NKC], fp8, kind="ExternalInput")
    vcC = nc.dram_tensor("vcC", [RPC, 128, H * KTC * VP], fp8, kind="ExternalInput")
    wqk = nc.dram_tensor("wqk", [8, 128, 2, CT, 128], fp8, kind="ExternalInput")
    wv = nc.dram_tensor("wv", [4, 128, CT, 256], fp8, kind="ExternalInput")
    wp = nc.dram_tensor("wp", [CT, 128, CT, 128], fp8, kind="ExternalInput")
    w1 = nc.dram_tensor("w1", [NHT // 2, 128, 2, 2, CT, 128], fp8,
                        kind="ExternalInput")
    w2 = nc.dram_tensor("w2", [CT, 128, 2, NHT, 128], fp8, kind="ExternalInput")
    if with_bias:
        wxp = nc.dram_tensor("wxp", [1, CT, 2, 128], fp8, kind="ExternalInput")
        wx1 = nc.dram_tensor("wx1", [1, NHT, 2, 128], fp8, kind="ExternalInput")
        wx2 = nc.dram_tensor("wx2", [1, CT, 2, 128], fp8, kind="ExternalInput")
    bqk = nc.dram_tensor("bqk", [128, 16], f32, kind="ExternalInput")
    vbias = nc.dram_tensor("vbias", [C], f32, kind="ExternalInput")
    ones = nc.dram_tensor("ones", [128, 1], f32r, kind="ExternalInput")
    outT = nc.dram_tensor("outT", [RPC, 128, CT, T], f32, kind="ExternalOutput")

    from contextlib import ExitStack
    with nc.allow_low_precision(reason="deliberate fp8/f32r staging; PSUM accumulation fp32"), \
         tile.TileContext(nc, pool_alloc_mode="queue") as tc, ExitStack() as es:
        # ---------- constants ----------
        consts = es.enter_context(tc.tile_pool(name="consts", bufs=1))
        ones_sb = consts.tile([128, 1], f32r)
        nc.sync.dma_start(ones_sb[:], ones.ap())
        ones_bf = consts.tile([128, 1], bf16)
        nc.vector.memset(ones_bf[:], 1.0)
        ones8 = consts.tile([128, 2, 16], fp8)
        nc.vector.memset(ones8[:], 1.0)
        bqk_sb = consts.tile([128, 16], f32)
        nc.sync.dma_start(bqk_sb[:], bqk.ap())
        if with_bias:
            vb_sb = consts.tile([128, C], bf16)
            nc.gpsimd.dma_start(vb_sb[:], vbias.ap()[None].to_broadcast((128, C)))
        if with_bias:
            wxp_sb = consts.tile([1, CT, 2, 128], fp8)
            nc.sync.dma_start(wxp_sb[:], wxp.ap())
            wx1_sb = consts.tile([1, NHT, 2, 128], fp8)
            nc.sync.dma_start(wx1_sb[:], wx1.ap())
            wx2_sb = consts.tile([1, CT, 2, 128], fp8)
            nc.sync.dma_start(wx2_sb[:], wx2.ap())
            xtr_p = consts.tile([1, 2, 512], fp8)
            nc.vector.memset(xtr_p[0:1, 0, :], OS)
            nc.vector.memset(xtr_p[0:1, 1, :], 0.0)
            xtr_1 = consts.tile([1, 2, 512], fp8)
            nc.vector.memset(xtr_1[0:1, 0, :], 1.0)
            nc.vector.memset(xtr_1[0:1, 1, :], 0.0)
        eps_sb = consts.tile([1, 1], f32)
        nc.vector.memset(eps_sb[:], EPS)

        def open_pool(nm, space=None):
            kw = dict(space=space) if space else {}
            cm = tc.tile_pool(name=nm, bufs=1, **kw)
            return cm, cm.__enter__()

        def close_pool(cm):
            cm.__exit__(None, None, None)

        # ---------- long-lived pools (ring-stack order) ----------
        cm_work, p_work = open_pool("p_work")
        cm_xs, p_xs = open_pool("p_xs")
        xs = [p_xs.tile([128, CT, T], f32r, tag=f"xs{r}", name=f"xs{r}")
              for r in range(RPC)]
        cm_oT, p_oT = open_pool("p_oT")
        oT = [p_oT.tile([128, CT, T], fp8, tag=f"oT{r}", name=f"oT{r}")
              for r in range(RPC)]
        cm_x2, p_x2 = open_pool("p_x2")
        x2s = [p_x2.tile([128, CT, T], bf16, tag=f"x2{r}", name=f"x2{r}")
               for r in range(RPC)]
        cm_g, p_g = open_pool("p_g")
        gs = [p_g.tile([128, NHT, T], fp8, tag=f"g{r}", name=f"g{r}")
              for r in range(RPC)]
        cm_h2, p_h2 = open_pool("p_h2")
        h2 = [p_h2.tile([128, 2, CT, T], fp8, tag=f"h2{r}", name=f"h2{r}")
              for r in range(RPC)]
        cm_kv, p_kv = open_pool("p_kv")
        kc_sb = [p_kv.tile([128, 2, 4, NKC], fp8, tag=f"kc{r}", name=f"kc{r}")
                 for r in range(RPC)]
        vkv = [p_kv.tile([128, 2, H, KTC, VP], fp8, tag=f"vkv{r}", name=f"vkv{r}")
               for r in range(RPC)]
        qD = [p_kv.tile([128, 2, 4, T], fp8, tag=f"qD{r}", name=f"qD{r}")
              for r in range(RPC)]
        kD = [p_kv.tile([128, 2, 4, T], fp8, tag=f"kD{r}", name=f"kD{r}")
              for r in range(RPC)]
        cm_att, apool = open_pool("p_att")
        cm_psatt, psatt = open_pool("ps_att", space="PSUM")
        cm_h1, p_h1 = open_pool("p_h1")
        h1 = [p_h1.tile([128, CT, T], fp8, tag=f"h1{r}", name=f"h1{r}")
              for r in range(RPC)]

        # ================= layernorm =================
        def layernorm(src, dst, stats_pool, st_tag, lhs_ones, src_cast,
                      dst_lo=None):
            s_t = stats_pool.tile([128, T], f32, tag="misc",
                                  name=f"{st_tag}s", bufs=2)
            s_ps = s_t[0:1, :]
            for ct in range(CT):
                nc.tensor.matmul(s_ps, lhs_ones, src[:, ct, :],
                                 start=(ct == 0), stop=(ct == CT - 1))
            ss_t = stats_pool.tile([128, T], f32, tag="misc",
                                   name=f"{st_tag}ss", bufs=2)
            ss_ps = ss_t[0:1, :]
            for c in range(CT // 2):
                sqs = p_work.tile([128, 2, T], fp8, tag="sqs", name="sqs", bufs=1)
                for j in range(2):
                    eng = nc.gpsimd if j else nc.vector
                    eng.tensor_mul(sqs[:, j, :], src_cast(src[:, 2 * c + j, :]),
                                   src_cast(src[:, 2 * c + j, :]))
                nc.tensor.matmul(ss_ps, ones8[:, :, 0:1], sqs[:],
                                 start=(c == 0), stop=(c == CT // 2 - 1),
                                 perf_mode=DRm)
            st = p_work.tile([97, T], f32, tag="st", name="st", bufs=1)
            negmu, msq, var, stdv = (st[0:1, :], st[32:33, :], st[64:65, :],
                                     st[96:97, :])
            nc.vector.tensor_scalar(negmu, s_ps, -1.0 / C, None, OP.mult)
            nc.vector.tensor_mul(msq, negmu, negmu)
            nc.vector.scalar_tensor_tensor(var, ss_ps, 1.0 / C, msq,
                                           OP.mult, OP.subtract)
            nc.scalar.activation(stdv, var, AF.Sqrt, bias=eps_sb[:])
            rstd = p_work.tile([1, T], f32, tag="rstd", name="rstd", bufs=2)
            nc.vector.reciprocal(rstd[:], stdv)
            A_sb = p_work.tile([128, T], f32, tag="Asb", name="Asb", bufs=1)
            nc.gpsimd.partition_broadcast(A_sb[:], rstd[:])
            M_sb = p_work.tile([128, T], f32, tag="Msb", name="Msb", bufs=1)
            nc.gpsimd.partition_broadcast(M_sb[:], negmu)
            for ct in range(CT):
                tmp = p_work.tile([128, T], f32, tag="tmp", name="tmp", bufs=2)
                e1, e2 = ((nc.vector, nc.gpsimd) if ct % 2 == 0
                          else (nc.gpsimd, nc.vector))
                e1.tensor_add(tmp[:], src_cast(src[:, ct, :]), M_sb[:])
                if dst_lo is None:
                    e2.tensor_mul(dst[:, ct, :], tmp[:], A_sb[:])
                else:
                    e2.tensor_mul(tmp[:], tmp[:], A_sb[:])
                    e1.tensor_copy(dst[:, ct, :], tmp[:])
                    nc.vector.scalar_tensor_tensor(
                        dst_lo[:, ct, :], dst[:, ct, :], -1.0, tmp[:],
                        OP.mult, OP.add)

        # ================= emitters =================
        def emit_qk_pair(f, r, wpool, mmps, qsb, ksb):
            wt = wpool.tile([128, 2, CT, 128], fp8, tag="wqk", name="wqk", bufs=3)
            nc.sync.dma_start(wt[:], wqk.ap()[f])
            for which in range(2):
                ps = mmps.tile([128, T], f32, tag="mm", name="mm", bufs=2)
                for c in range(CT // 2):
                    nc.tensor.matmul(ps[:], wt[:, which, 2 * c:2 * c + 2, :],
                                     h1[r][:, 2 * c:2 * c + 2, :],
                                     start=(c == 0), stop=(c == CT // 2 - 1),
                                     perf_mode=DRm)
                dst = (qsb if which == 0 else ksb)[:, f, :]
                nc.vector.tensor_scalar(dst, ps[:], 1.0 / WS,
                                        bqk_sb[:, 8 * which + f:8 * which + f + 1],
                                        OP.mult, OP.add)

        def emit_v_chunk(ch, r, wpool, mmps):
            wvt = wpool.tile([128, CT, 256], fp8, tag="wv", name="wv", bufs=2)
            nc.sync.dma_start(wvt[:], wv.ap()[ch])
            for tt in range(KTN):
                psf = mmps.tile([128, T], f32, tag="mm", name="mm", bufs=2)
                ps = psf[:, 0:256]
                for c in range(CT // 2):
                    nc.tensor.matmul(
                        ps, h1[r][:, 2 * c:2 * c + 2, tt * 128:(tt + 1) * 128],
                        wvt[:, 2 * c:2 * c + 2, :],
                        start=(c == 0), stop=(c == CT // 2 - 1), perf_mode=DRm)
                if with_bias:
                    nc.vector.scalar_tensor_tensor(
                        vkv[r][:, 1, 4 * ch:4 * ch + 4, tt, 0:HD],
                        ps.rearrange("p (h d) -> p h d", h=4), OS / WS,
                        vb_sb[:, ch * 256:(ch + 1) * 256]
                        .rearrange("p (h d) -> p h d", h=4),
                        OP.mult, OP.add)
                else:
                    nc.vector.tensor_scalar(
                        vkv[r][:, 1, 4 * ch:4 * ch + 4, tt, 0:HD],
                        ps.rearrange("p (h d) -> p h d", h=4), OS / WS,
                        None, OP.mult)

        def emit_swizzle(r, qsb, ksb, fq):
            # head h=2f+par -> block d=2*par+f//4, hg=f%4
            for src_sb, dst in ((qsb, qD[r]), (ksb, kD[r])):
                for par in range(2):
                    for half in range(2):
                        nc.sync.dma_start(
                            dst[32 * (2 * par + fq):32 * (2 * par + fq) + 32,
                                half, :, :],
                            src_sb[64 * par + 32 * half:
                                   64 * par + 32 * half + 32,
                                   4 * fq:4 * fq + 4, :])

        def emit_attention(hp, r):
            for hh in range(2):
                h = 2 * hp + hh
                d = 2 * (h % 2) + (h // 2) // 4
                hg = (h // 2) % 4
                pb = 32 * d
                pt = apool.tile([128, KT, T], fp8, tag="pt", name="pt", bufs=2)
                pv = psatt.tile([128, T], f32, tag="pv", name="pv", bufs=2)
                for g in range(KT // 2):
                    sc = psatt.tile([128, 2, T], f32, tag="sc", name="sc", bufs=2)
                    for j in range(2):
                        kt = 2 * g + j
                        if kt < KTC:
                            lhs = kc_sb[r][pb:pb + 32, :, hg,
                                           kt * 128:(kt + 1) * 128]
                        else:
                            lhs = kD[r][pb:pb + 32, :, hg,
                                        (kt - KTC) * 128:(kt - KTC + 1) * 128]
                        nc.tensor.matmul(sc[:, j, :], lhs,
                                         qD[r][pb:pb + 32, :, hg, :],
                                         start=True, stop=True, perf_mode=DRm,
                                         tile_position=(pb, 0))
                    nc.scalar.activation(
                        pt[:, 2 * g:2 * g + 2, :].rearrange("p a b -> p (a b)"),
                        sc[:].rearrange("p a b -> p (a b)"), AF.Exp, scale=SCALE)
                    half = 0 if g < KTC // 2 else 1
                    koff = 0 if g < KTC // 2 else KTC
                    nc.tensor.matmul(
                        pv[0:VP, :],
                        vkv[r][:, half, h, 2 * g - koff:2 * g - koff + 2, :],
                        pt[:, 2 * g:2 * g + 2, :],
                        start=(g == 0), stop=(g == KT // 2 - 1), perf_mode=DRm)
                rd = apool.tile([1, T], f32, tag="rd", name="rd", bufs=2)
                nc.vector.reciprocal(rd[:], pv[HD:HD + 1, :])
                bcs = apool.tile([HD, T], f32, tag="bcs", name="bcs", bufs=2)
                nc.gpsimd.partition_broadcast(bcs[:], rd[:])
                nc.vector.tensor_mul(oT[r][64 * hh:64 * hh + 64, hp, :],
                                     pv[0:HD, :], bcs[:])

        def emit_proj(co, r, wpool, mpool):
            wt = wpool.tile([128, CT, 128], fp8, tag="wp", name="wp", bufs=3)
            nc.sync.dma_start(wt[:], wp.ap()[co])
            ps = mpool.tile([128, T], f32, tag="misc", name="misc", bufs=2)
            for c in range(CT // 2):
                nc.tensor.matmul(ps[:], wt[:, 2 * c:2 * c + 2, :],
                                 oT[r][:, 2 * c:2 * c + 2, :],
                                 start=(c == 0),
                                 stop=(not with_bias and c == CT // 2 - 1),
                                 perf_mode=DRm)
            if with_bias:
                nc.tensor.matmul(ps[:], wxp_sb[0:1, co, :, :], xtr_p[0:1, :, :],
                                 start=False, stop=True, perf_mode=DRm)
            nc.vector.scalar_tensor_tensor(
                x2s[r][:, co, :], ps[:], 1.0 / (OS * WS),
                xs[r][:, co, :].bitcast(f32), OP.mult, OP.add)

        def emit_fc1_pair(htp, rows, wpool, mpool):
            wt = wpool.tile([128, 2, 2, CT, 128], fp8, tag="w1", name="w1", bufs=3)
            nc.sync.dma_start(wt[:], w1.ap()[htp])
            for r in rows:
                for j in range(2):
                    ht = 2 * htp + j
                    ps = mpool.tile([128, T], f32, tag="misc", name="misc", bufs=2)
                    for pi, (pw, ph) in enumerate(((0, 0), (0, 1), (1, 0))):
                        for c in range(CT // 2):
                            nc.tensor.matmul(ps[:],
                                             wt[:, j, pw, 2 * c:2 * c + 2, :],
                                             h2[r][:, ph, 2 * c:2 * c + 2, :],
                                             start=(pi == 0 and c == 0),
                                             stop=(not with_bias and pi == 2
                                                   and c == CT // 2 - 1),
                                             perf_mode=DRm)
                    if with_bias:
                        nc.tensor.matmul(ps[:], wx1_sb[0:1, ht, :, :],
                                         xtr_1[0:1, :, :],
                                         start=False, stop=True, perf_mode=DRm)
                    nc.scalar.activation(gs[r][:, ht, :], ps[:], AF.Gelu,
                                         scale=1.0 / WS)

        def emit_fc2(co, r, wt, f2pool, opool):
            ps = f2pool.tile([128, T], f32, tag="fc2", name="fc2", bufs=3)
            for pw in range(2):
                for tp in range(NHT // 2):
                    nc.tensor.matmul(ps[:], wt[:, pw, 2 * tp:2 * tp + 2, :],
                                     gs[r][:, 2 * tp:2 * tp + 2, :],
                                     start=(pw == 0 and tp == 0),
                                     stop=(not with_bias and pw == 1
                                           and tp == NHT // 2 - 1),
                                     perf_mode=DRm)
            if with_bias:
                nc.tensor.matmul(ps[:], wx2_sb[0:1, co, :, :], xtr_1[0:1, :, :],
                                 start=False, stop=True, perf_mode=DRm)
            ot = opool.tile([128, T], f32, tag="ot", name="ot", bufs=3)
            nc.vector.scalar_tensor_tensor(
                ot[:], ps[:], 1.0 / WS, x2s[r][:, co, :], OP.mult, OP.add)
            nc.sync.dma_start(outT.ap()[r, :, co, :], ot[:])

        # ================= phase 0: loads + LN1 =================
        cm_psln, psln = open_pool("ps_ln", space="PSUM")
        for r in range(RPC):
            nc.sync.dma_start(xs[r][:, 0:CT // 2, :], xT.ap()[r, :, 0:CT // 2, :])
            nc.sync.dma_start(xs[r][:, CT // 2:, :], xT.ap()[r, :, CT // 2:, :])
            nc.sync.dma_start(
                kc_sb[r][:].rearrange("p a b c -> p (a b c)"), kcD.ap()[r])
            nc.sync.dma_start(
                vkv[r][:, 0, :, :, :].rearrange("p h k d -> p (h k d)"),
                vcC.ap()[r])
            nc.gpsimd.memset(vkv[r][:, 1, :, :, HD:HD + 1], 1.0)
            nc.gpsimd.memset(vkv[r][:, 1, :, :, HD + 1:VP], 0.0)
        for r in range(RPC):
            layernorm(xs[r], h1[r], psln, "l1", ones_sb[:, 0:1],
                      lambda ap: ap.bitcast(f32))
        close_pool(cm_psln)

        # ================= qkv r0 + swizzle r0 =================
        cm_wA, wA = open_pool("p_wA")
        cm_psmm, psmm = open_pool("ps_mm", space="PSUM")
        cm_st0, p_st0 = open_pool("p_st0")
        q0 = p_st0.tile([128, CT, T], fp8, tag="q0", name="q0")
        k0 = p_st0.tile([128, CT, T], fp8, tag="k0", name="k0")
        for f in range(4):
            emit_qk_pair(f, 0, wA, psmm, q0, k0)
        emit_v_chunk(0, 0, wA, psmm)
        emit_v_chunk(1, 0, wA, psmm)
        emit_swizzle(0, q0, k0, 0)
        for f in range(4, 8):
            emit_qk_pair(f, 0, wA, psmm, q0, k0)
        emit_v_chunk(2, 0, wA, psmm)
        emit_v_chunk(3, 0, wA, psmm)
        emit_swizzle(0, q0, k0, 1)
        close_pool(cm_st0)

        # ================= phase A: attention r0 || qkv r1 =================
        cm_st1, p_st1 = open_pool("p_st1")
        q1 = p_st1.tile([128, CT, T], fp8, tag="q1", name="q1")
        k1 = p_st1.tile([128, CT, T], fp8, tag="k1", name="k1")
        for hp in range(HPAIR):
            emit_attention(hp, 0)
        for f in range(8):
            emit_qk_pair(f, 1, wA, psmm, q1, k1)
            if f % 2 == 1:
                emit_v_chunk(f // 2, 1, wA, psmm)
            if f == 3:
                emit_swizzle(1, q1, k1, 0)
        emit_swizzle(1, q1, k1, 1)
        close_pool(cm_st1)
        close_pool(cm_psmm)
        close_pool(cm_wA)
        close_pool(cm_h1)

        # ====== phase B: attention r1 (priority) || proj/LN2/FC1 r0 ======
        cm_psB, psB = open_pool("ps_B", space="PSUM")
        cm_wB, wB = open_pool("p_wB")
        for hp in range(HPAIR):
            emit_attention(hp, 1)
        for co in range(CT):
            emit_proj(co, 0, wB, psB)
        layernorm(x2s[0], h2[0][:, 0], psB, "l2a", ones_bf[:],
                  lambda ap: ap, dst_lo=h2[0][:, 1])
        for htp in range(NHT // 2):
            emit_fc1_pair(htp, [0], wB, psB)
        close_pool(cm_wB)
        close_pool(cm_psB)
        close_pool(cm_psatt)
        close_pool(cm_att)
        close_pool(cm_kv)

        # ================= tail =================
        with tc.tile_pool(name="ps_T", space="PSUM", bufs=1) as psT, \
             tc.tile_pool(name="p_wT", bufs=1) as wT, \
             tc.tile_pool(name="p_osb", bufs=1) as osb:
            for co in range(CT):
                emit_proj(co, 1, wT, psT)
            layernorm(x2s[1], h2[1][:, 0], psT, "l2b", ones_bf[:],
                      lambda ap: ap, dst_lo=h2[1][:, 1])
            for co in range(CT):
                w2t = wT.tile([128, 2, NHT, 128], fp8, tag="w2", name="w2", bufs=2)
                nc.sync.dma_start(w2t[:], w2.ap()[co])
                emit_fc2(co, 0, w2t, psT, osb)
            for htp in range(NHT // 2):
                emit_fc1_pair(htp, [1], wT, psT)
            for co in range(CT):
                w2t = wT.tile([128, 2, NHT, 128], fp8, tag="w2", name="w2", bufs=2)
                nc.sync.dma_start(w2t[:], w2.ap()[co])
                emit_fc2(co, 1, w2t, psT, osb)
        close_pool(cm_h2)
        close_pool(cm_g)
        close_pool(cm_x2)
        close_pool(cm_oT)
        close_pool(cm_xs)
        close_pool(cm_work)

    nc.compile()
    return nc


class _Runner:
    """Hold the compiled PJRT executable (mirrors bass2jax.run_bass_via_pjrt)."""

    def __init__(self, nc, n_cores):
        import jax
        from jax.sharding import Mesh, PartitionSpec
        from jax.experimental.shard_map import shard_map
        import concourse.mybir as mybir
        from concourse.bass2jax import (
            install_neuronx_cc_hook, partition_id_tensor, _bass_exec_p)

        install_neuronx_cc_hook()
        self.jax = jax
        self.n_cores = n_cores
        partition_name = nc.partition_id_tensor.name if nc.partition_id_tensor else None
        in_names, out_names, out_avals, zero_outs = [], [], [], []
        for alloc in nc.m.functions[0].allocations:
            if not isinstance(alloc, mybir.MemoryLocationSet):
                continue
            name = alloc.memorylocations[0].name
            if alloc.kind == "ExternalInput":
                if name != partition_name:
                    in_names.append(name)
            elif alloc.kind == "ExternalOutput":
                shape = tuple(alloc.tensor_shape)
                dtype = mybir.dt.np(alloc.dtype)
                out_names.append(name)
                out_avals.append(jax.core.ShapedArray(shape, dtype))
                zero_outs.append(np.zeros(shape, dtype))
        self.in_names, self.out_names = in_names, out_names
        self.out_avals, self.zero_outs = out_avals, zero_outs
        self.n_params = len(in_names)
        all_names = in_names + out_names
        if partition_name is not None:
            all_names.append(partition_name)

        def _body(*args):
            operands = list(args)
            if partition_name is not None:
                operands.append(partition_id_tensor())
            return tuple(
                _bass_exec_p.bind(
                    *operands,
                    out_avals=tuple(out_avals),
                    in_names=tuple(all_names),
                    out_names=tuple(out_names),
                    lowering_input_output_aliases=(),
                    sim_require_finite=True,
                    sim_require_nnan=True,
                    nc=nc,
                ))

        devices = jax.devices()[:n_cores]
        assert len(devices) == n_cores, f"need {n_cores} cores, have {len(jax.devices())}"
        mesh = Mesh(np.asarray(devices), ("core",))
        n_outs = len(out_names)
        self._fn = jax.jit(
            shard_map(_body, mesh=mesh,
                      in_specs=(PartitionSpec("core"),) * (self.n_params + n_outs),
                      out_specs=(PartitionSpec("core"),) * n_outs,
                      check_rep=False),
            keep_unused=True)

    def prepare(self, in_maps):
        np_ = np
        per_core = [[np_.asarray(m[n]) for n in self.in_names] for m in in_maps]
        concat_in = [
            np_.concatenate([per_core[c][i] for c in range(self.n_cores)], axis=0)
            for i in range(self.n_params)]
        concat_zeros = [
            np_.zeros((self.n_cores * z.shape[0], *z.shape[1:]), z.dtype)
            for z in self.zero_outs]
        return self.jax.device_put(concat_in + concat_zeros)

    def run(self, prepared):
        out = self._fn(*prepared)
        self.jax.block_until_ready(out)
        return out

    def results(self, out_arrs):
        return [
            {name: np.asarray(out_arrs[i]).reshape(
                self.n_cores, *self.out_avals[i].shape)[c]
             for i, name in enumerate(self.out_names)}
            for c in range(self.n_cores)]


def _get_runner(with_bias):
    if "runner" not in _state:
        nc = _build_module(with_bias=with_bias)
        _state["nc"] = nc
        _state["runner"] = _Runner(nc, NCORES)
    return _state["runner"]


def _prepare_in_maps(x, cache_k, cache_v, update_mask, qkv_w, qkv_b, proj_w,
                     proj_b, n1_g, n1_b, n2_g, n2_b, fc1_w, fc1_b, fc2_w, fc2_b):
    f32 = np.float32
    x = np.asarray(x, f32)
    cache_k = np.asarray(cache_k, f32)
    cache_v = np.asarray(cache_v, f32)
    update_mask = np.asarray(update_mask, bool)
    qkv_w = np.asarray(qkv_w, f32)
    qkv_b = np.asarray(qkv_b, f32)
    proj_w = np.asarray(proj_w, f32)
    proj_b = np.asarray(proj_b, f32)
    n1_g = np.asarray(n1_g, f32)
    n1_b = np.asarray(n1_b, f32)
    n2_g = np.asarray(n2_g, f32)
    n2_b = np.asarray(n2_b, f32)
    fc1_w = np.asarray(fc1_w, f32)
    fc1_b = np.asarray(fc1_b, f32)
    fc2_w = np.asarray(fc2_w, f32)
    fc2_b = np.asarray(fc2_b, f32)

    xT = np.ascontiguousarray(
        x.transpose(0, 2, 1).reshape(B, CT, 128, T).transpose(0, 2, 1, 3))

    kcD = np.empty((B, 128, 2, 4, NKC), fp8np)
    vcC = np.empty((B, 128, H, KTC, VP), fp8np)
    for b in range(B):
        keep = ~update_mask[b]
        kc = cache_k[b][:, keep, :]          # [H, NKC, HD]
        vc = cache_v[b][:, keep, :]
        kt_ = kc.transpose(0, 2, 1)          # [H, HD, NKC]
        for h in range(H):
            d_ = 2 * (h % 2) + (h // 2) // 4
            hg_ = (h // 2) % 4
            for half in range(2):
                kcD[b, 32 * d_:32 * d_ + 32, half, hg_, :] = (
                    kt_[h, 32 * half:32 * half + 32, :].astype(fp8np))
        vv = (vc.transpose(1, 0, 2).reshape(KTC, 128, H, HD)
              .transpose(1, 2, 0, 3))        # [128, H, KTC, HD]
        pad = np.zeros((128, H, KTC, VP - HD), f32)
        pad[:, :, :, 0] = 1.0
        vcC[b] = np.concatenate([vv * OS, pad], axis=3).astype(fp8np)

    def wtile(w, nf, cols):
        ci = w.shape[0]
        return np.ascontiguousarray(
            (WS * w).reshape(ci // 128, 128, nf, cols)
            .transpose(2, 1, 0, 3)).astype(fp8np)

    def wtile_hl(w, nf, cols):
        ws = WS * w
        hi = ws.astype(fp8np)
        lo = (ws - hi.astype(np.float32)).astype(fp8np)
        ci = w.shape[0]

        def t(a):
            return (a.reshape(ci // 128, 128, nf, cols).transpose(2, 1, 0, 3))
        return np.ascontiguousarray(
            np.stack([t(hi), t(lo)], axis=2)).astype(fp8np)

    wqkv_eff = n1_g[:, None] * qkv_w
    bias_qkv = n1_b @ qkv_w + qkv_b
    wqk16 = wtile(wqkv_eff[:, :2048], 16, 128)       # [16,128,CT,128]
    wqk_t = np.empty((8, 128, 2, CT, 128), fp8np)
    for f in range(8):
        wqk_t[f, :, 0] = wqk16[f]
        wqk_t[f, :, 1] = wqk16[8 + f]
    wv_t = wtile(wqkv_eff[:, 2048:], 4, 256)
    bqk_t = np.ascontiguousarray(bias_qkv[:2048].reshape(16, 128).T).astype(f32)
    vbias_t = (OS * bias_qkv[2048:]).astype(f32)

    wp_t = wtile(proj_w, CT, 128)
    wxp_t = np.zeros((1, CT, 2, 128), fp8np)
    wxp_t[0, :, 0, :] = (WS * proj_b).reshape(CT, 128).astype(fp8np)

    w1_eff = n2_g[:, None] * fc1_w
    bias_fc1 = n2_b @ fc1_w + fc1_b
    w1_hl = wtile_hl(w1_eff, NHT, 128)               # [32,128,2,CT,128]
    w1_t = np.ascontiguousarray(
        w1_hl.reshape(NHT // 2, 2, 128, 2, CT, 128)
        .transpose(0, 2, 1, 3, 4, 5))                # [16,128,2,2,CT,128]
    wx1_t = np.zeros((1, NHT, 2, 128), fp8np)
    wx1_t[0, :, 0, :] = (WS * bias_fc1).reshape(NHT, 128).astype(fp8np)

    w2_t = wtile_hl(fc2_w, CT, 128)                  # [CT,128,2,NHT,128]
    wx2_t = np.zeros((1, CT, 2, 128), fp8np)
    wx2_t[0, :, 0, :] = (WS * fc2_b).reshape(CT, 128).astype(fp8np)

    with_bias = bool(np.any(wxp_t) or np.any(wx1_t) or np.any(wx2_t))
    shared = dict(
        wqk=wqk_t, wv=wv_t, wp=wp_t, w1=w1_t, w2=w2_t,
        bqk=bqk_t, vbias=vbias_t,
        ones=np.ones((128, 1), f32),
    )
    if with_bias:
        shared.update(wxp=wxp_t, wx1=wx1_t, wx2=wx2_t)
    in_maps = []
    for c in range(NCORES):
        s = slice(c * RPC, (c + 1) * RPC)
        in_maps.append(dict(
            shared, xT=xT[s], kcD=kcD[s],
            vcC=vcC[s].reshape(RPC, 128, H * KTC * VP)))
    return in_maps, with_bias


def kernel(**inputs) -> np.ndarray:
    in_maps, with_bias = _prepare_in_maps(**inputs)
    runner = _get_runner(with_bias)
    prepared = runner.prepare(in_maps)
    out = runner.run(prepared)
    res = runner.results(out)
    full = np.empty((B, NP, C), np.float32)
    for c in range(NCORES):
        for r in range(RPC):
            full[c * RPC + r] = res[c]["outT"][r].transpose(2, 1, 0).reshape(T, C)
    return full


# revision 27
# speedup vs baseline: 1.7660x; 1.0763x over previous
"""Fused decoder block (LN->QKV->cache-merge attention->proj->LN->MLP) on 8
Trainium2 NeuronCores, data-parallel over the batch (2 rows/core).

v3: row-pipelined schedule. The two batch rows per core are independent, so
the ACT-bound attention of row 1 overlaps the PE-bound proj/LN2/FC1 of row 0.

Key ideas (cumulative):
- host-side cache compaction via update_mask: attention over [512 surviving
  cache keys ++ 512 new keys] = 1024 keys, no mask bias (softmax is
  permutation invariant).
- fp8e4 DoubleRow matmuls everywhere (QKV, scores, PV, proj, FC1, FC2);
  weights host-quantized at x64 scale; hi+lo residual passes for W1/W2/h2
  keep the MLP path accurate (rel err ~1.3e-2 < 2e-2).
- q/k swizzled into [32, 2, head, T] pair layout by SBUF->SBUF DMAs so the
  64-dim head contraction runs as DoubleRow (half cost).
- LN standardization only (gains/biases folded into weights host-side);
  sum-sq stats via fp8 DR on squared tiles; rstd/-mu broadcast across
  partitions by GPSIMD partition_broadcast (no PE, no PSUM).
- linear biases as extra fp8 contraction rows; softmax denominator from a
  ones column in V, reciprocal broadcast on Pool.
- x2 residual kept in bf16 to fit SBUF; PSUM pools sized to exactly 8 banks
  per phase.
"""

import numpy as np
import ml_dtypes

B, NP, N, C, H = 16, 512, 1024, 1024, 16
HD = C // H            # 64
HID = 4 * C            # 4096
EPS = 1e-5
NCORES = 8
RPC = B // NCORES      # batch rows per core
T = NP                 # queries per row
CT = C // 128          # feature tiles
NKC = N - NP           # surviving cache keys (512)
KTC = NKC // 128       # cache key tiles (4)
KTN = T // 128         # new key tiles (4)
KT = KTC + KTN         # total key tiles (8)
HPAIR = H // 2
NHT = HID // 128       # fc1 output chunks (32)
SCALE = HD ** -0.5
WS = 64.0              # weight quantization scale
OS = 16.0              # v / attention-output scale
VP = 80                # padded v row (64 d + 1 ones + 15 pad)
W2P = 1                # fc2 weight passes (1 = hi only, 2 = hi+lo residual)

_state = {}
fp8np = ml_dtypes.float8_e4m3


def _build_module(with_bias=True):
    import concourse.tile as tile
    from concourse import bacc, mybir

    f32 = mybir.dt.float32
    f32r = mybir.dt.float32r
    bf16 = mybir.dt.bfloat16
    fp8 = mybir.dt.float8e4
    AF = mybir.ActivationFunctionType
    OP = mybir.AluOpType
    DRm = mybir.MatmulPerfMode.DoubleRow

    nc = bacc.Bacc("TRN2", target_bir_lowering=False, debug=False)

    xT = nc.dram_tensor("xT", [RPC, 128, CT, T], f32r, kind="ExternalInput")
    kcD = nc.dram_tensor("kcD", [RPC, 128, 2, 4, NKC], fp8, kind="ExternalInput")
    vcC = nc.dram_tensor("vcC", [RPC, 128, H * KTC * VP], fp8, kind="ExternalInput")
    wqk = nc.dram_tensor("wqk", [8, 128, 2, CT, 128], fp8, kind="ExternalInput")
    wv = nc.dram_tensor("wv", [4, 128, CT, 256], fp8, kind="ExternalInput")
    wp = nc.dram_tensor("wp", [CT, 128, CT, 128], fp8, kind="ExternalInput")
    w1 = nc.dram_tensor("w1", [NHT // 2, 128, 2, 2, CT, 128], fp8,
                        kind="ExternalInput")
    w2 = nc.dram_tensor("w2", [CT, 128, W2P, NHT, 128], fp8, kind="ExternalInput")
    if with_bias:
        wxp = nc.dram_tensor("wxp", [1, CT, 2, 128], fp8, kind="ExternalInput")
        wx1 = nc.dram_tensor("wx1", [1, NHT, 2, 128], fp8, kind="ExternalInput")
        wx2 = nc.dram_tensor("wx2", [1, CT, 2, 128], fp8, kind="ExternalInput")
    bqk = nc.dram_tensor("bqk", [128, 16], f32, kind="ExternalInput")
    vbias = nc.dram_tensor("vbias", [C], f32, kind="ExternalInput")
    ones = nc.dram_tensor("ones", [128, 1], f32r, kind="ExternalInput")
    outT = nc.dram_tensor("outT", [RPC, 128, CT, T], f32, kind="ExternalOutput")

    from contextlib import ExitStack
    with nc.allow_low_precision(reason="deliberate fp8/f32r staging; PSUM accumulation fp32"), \
         tile.TileContext(nc, pool_alloc_mode="queue") as tc, ExitStack() as es:
        # ---------- constants ----------
        consts = es.enter_context(tc.tile_pool(name="consts", bufs=1))
        ones_sb = consts.tile([128, 1], f32r)
        nc.sync.dma_start(ones_sb[:], ones.ap())
        ones_bf = consts.tile([128, 1], bf16)
        nc.vector.memset(ones_bf[:], 1.0)
        ones8 = consts.tile([128, 2, 16], fp8)
        nc.vector.memset(ones8[:], 1.0)
        bqk_sb = consts.tile([128, 16], f32)
        nc.sync.dma_start(bqk_sb[:], bqk.ap())
        if with_bias:
            vb_sb = consts.tile([128, C], bf16)
            nc.gpsimd.dma_start(vb_sb[:], vbias.ap()[None].to_broadcast((128, C)))
        if with_bias:
            wxp_sb = consts.tile([1, CT, 2, 128], fp8)
            nc.sync.dma_start(wxp_sb[:], wxp.ap())
            wx1_sb = consts.tile([1, NHT, 2, 128], fp8)
            nc.sync.dma_start(wx1_sb[:], wx1.ap())
            wx2_sb = consts.tile([1, CT, 2, 128], fp8)
            nc.sync.dma_start(wx2_sb[:], wx2.ap())
            xtr_p = consts.tile([1, 2, 512], fp8)
            nc.vector.memset(xtr_p[0:1, 0, :], OS)
            nc.vector.memset(xtr_p[0:1, 1, :], 0.0)
            xtr_1 = consts.tile([1, 2, 512], fp8)
            nc.vector.memset(xtr_1[0:1, 0, :], 1.0)
            nc.vector.memset(xtr_1[0:1, 1, :], 0.0)
        eps_sb = consts.tile([1, 1], f32)
        nc.vector.memset(eps_sb[:], EPS)

        def open_pool(nm, space=None):
            kw = dict(space=space) if space else {}
            cm = tc.tile_pool(name=nm, bufs=1, **kw)
            return cm, cm.__enter__()

        def close_pool(cm):
            cm.__exit__(None, None, None)

        # ---------- long-lived pools (ring-stack order) ----------
        cm_work, p_work = open_pool("p_work")
        cm_xs, p_xs = open_pool("p_xs")
        xs = [p_xs.tile([128, CT, T], f32r, tag=f"xs{r}", name=f"xs{r}")
              for r in range(RPC)]
        cm_oT, p_oT = open_pool("p_oT")
        oT = [p_oT.tile([128, CT, T], fp8, tag=f"oT{r}", name=f"oT{r}")
              for r in range(RPC)]
        cm_x2, p_x2 = open_pool("p_x2")
        x2s = [p_x2.tile([128, CT, T], bf16, tag=f"x2{r}", name=f"x2{r}")
               for r in range(RPC)]
        cm_g, p_g = open_pool("p_g")
        gs = [p_g.tile([128, NHT, T], fp8, tag=f"g{r}", name=f"g{r}")
              for r in range(RPC)]
        cm_h2, p_h2 = open_pool("p_h2")
        h2 = [p_h2.tile([128, 2, CT, T], fp8, tag=f"h2{r}", name=f"h2{r}")
              for r in range(RPC)]
        cm_kv, p_kv = open_pool("p_kv")
        kc_sb = [p_kv.tile([128, 2, 4, NKC], fp8, tag=f"kc{r}", name=f"kc{r}")
                 for r in range(RPC)]
        vkv = [p_kv.tile([128, 2, H, KTC, VP], fp8, tag=f"vkv{r}", name=f"vkv{r}")
               for r in range(RPC)]
        qD = [p_kv.tile([128, 2, 4, T], fp8, tag=f"qD{r}", name=f"qD{r}")
              for r in range(RPC)]
        kD = [p_kv.tile([128, 2, 4, T], fp8, tag=f"kD{r}", name=f"kD{r}")
              for r in range(RPC)]
        cm_att, apool = open_pool("p_att")
        cm_psatt, psatt = open_pool("ps_att", space="PSUM")
        cm_h1, p_h1 = open_pool("p_h1")
        h1 = [p_h1.tile([128, CT, T], fp8, tag=f"h1{r}", name=f"h1{r}")
              for r in range(RPC)]

        # ================= layernorm =================
        def layernorm(src, dst, stats_pool, st_tag, lhs_ones, src_cast,
                      dst_lo=None):
            s_t = stats_pool.tile([128, T], f32, tag="misc",
                                  name=f"{st_tag}s", bufs=2)
            s_ps = s_t[0:1, :]
            for ct in range(CT):
                nc.tensor.matmul(s_ps, lhs_ones, src[:, ct, :],
                                 start=(ct == 0), stop=(ct == CT - 1))
            ss_t = stats_pool.tile([128, T], f32, tag="misc",
                                   name=f"{st_tag}ss", bufs=2)
            ss_ps = ss_t[0:1, :]
            for c in range(CT // 2):
                sqs = p_work.tile([128, 2, T], fp8, tag="sqs", name="sqs", bufs=1)
                for j in range(2):
                    eng = nc.gpsimd if j else nc.vector
                    eng.tensor_mul(sqs[:, j, :], src_cast(src[:, 2 * c + j, :]),
                                   src_cast(src[:, 2 * c + j, :]))
                nc.tensor.matmul(ss_ps, ones8[:, :, 0:1], sqs[:],
                                 start=(c == 0), stop=(c == CT // 2 - 1),
                                 perf_mode=DRm)
            st = p_work.tile([97, T], f32, tag="st", name="st", bufs=1)
            negmu, msq, var, stdv = (st[0:1, :], st[32:33, :], st[64:65, :],
                                     st[96:97, :])
            nc.vector.tensor_scalar(negmu, s_ps, -1.0 / C, None, OP.mult)
            nc.vector.tensor_mul(msq, negmu, negmu)
            nc.vector.scalar_tensor_tensor(var, ss_ps, 1.0 / C, msq,
                                           OP.mult, OP.subtract)
            nc.scalar.activation(stdv, var, AF.Sqrt, bias=eps_sb[:])
            rstd = p_work.tile([1, T], f32, tag="rstd", name="rstd", bufs=2)
            nc.vector.reciprocal(rstd[:], stdv)
            A_sb = p_work.tile([128, T], f32, tag="Asb", name="Asb", bufs=1)
            nc.gpsimd.partition_broadcast(A_sb[:], rstd[:])
            M_sb = p_work.tile([128, T], f32, tag="Msb", name="Msb", bufs=1)
            nc.gpsimd.partition_broadcast(M_sb[:], negmu)
            for ct in range(CT):
                tmp = p_work.tile([128, T], f32, tag="tmp", name="tmp", bufs=2)
                e1, e2 = ((nc.vector, nc.gpsimd) if ct % 2 == 0
                          else (nc.gpsimd, nc.vector))
                e1.tensor_add(tmp[:], src_cast(src[:, ct, :]), M_sb[:])
                if dst_lo is None:
                    e2.tensor_mul(dst[:, ct, :], tmp[:], A_sb[:])
                else:
                    e2.tensor_mul(tmp[:], tmp[:], A_sb[:])
                    e1.tensor_copy(dst[:, ct, :], tmp[:])
                    nc.vector.scalar_tensor_tensor(
                        dst_lo[:, ct, :], dst[:, ct, :], -1.0, tmp[:],
                        OP.mult, OP.add)

        # ================= emitters =================
        def emit_qk_pair(f, r, wpool, mmps, qsb, ksb):
            wt = wpool.tile([128, 2, CT, 128], fp8, tag="wqk", name="wqk", bufs=3)
            nc.sync.dma_start(wt[:], wqk.ap()[f])
            for which in range(2):
                ps = mmps.tile([128, T], f32, tag="mm", name="mm", bufs=2)
                for c in range(CT // 2):
                    nc.tensor.matmul(ps[:], wt[:, which, 2 * c:2 * c + 2, :],
                                     h1[r][:, 2 * c:2 * c + 2, :],
                                     start=(c == 0), stop=(c == CT // 2 - 1),
                                     perf_mode=DRm)
                dst = (qsb if which == 0 else ksb)[:, f, :]
                nc.vector.tensor_scalar(dst, ps[:], 1.0 / WS,
                                        bqk_sb[:, 8 * which + f:8 * which + f + 1],
                                        OP.mult, OP.add)

        def emit_v_chunk(ch, r, wpool, mmps):
            wvt = wpool.tile([128, CT, 256], fp8, tag="wv", name="wv", bufs=2)
            nc.sync.dma_start(wvt[:], wv.ap()[ch])
            for tt in range(KTN):
                psf = mmps.tile([128, T], f32, tag="mm", name="mm", bufs=2)
                ps = psf[:, 0:256]
                for c in range(CT // 2):
                    nc.tensor.matmul(
                        ps, h1[r][:, 2 * c:2 * c + 2, tt * 128:(tt + 1) * 128],
                        wvt[:, 2 * c:2 * c + 2, :],
                        start=(c == 0), stop=(c == CT // 2 - 1), perf_mode=DRm)
                if with_bias:
                    nc.vector.scalar_tensor_tensor(
                        vkv[r][:, 1, 4 * ch:4 * ch + 4, tt, 0:HD],
                        ps.rearrange("p (h d) -> p h d", h=4), OS / WS,
                        vb_sb[:, ch * 256:(ch + 1) * 256]
                        .rearrange("p (h d) -> p h d", h=4),
                        OP.mult, OP.add)
                else:
                    nc.vector.tensor_scalar(
                        vkv[r][:, 1, 4 * ch:4 * ch + 4, tt, 0:HD],
                        ps.rearrange("p (h d) -> p h d", h=4), OS / WS,
                        None, OP.mult)

        def emit_swizzle(r, qsb, ksb, fq):
            # head h=2f+par -> block d=2*par+f//4, hg=f%4
            for src_sb, dst in ((qsb, qD[r]), (ksb, kD[r])):
                for par in range(2):
                    for half in range(2):
                        nc.sync.dma_start(
                            dst[32 * (2 * par + fq):32 * (2 * par + fq) + 32,
                                half, :, :],
                            src_sb[64 * par + 32 * half:
                                   64 * par + 32 * half + 32,
                                   4 * fq:4 * fq + 4, :])

        def emit_attention(hp, r):
            for hh in range(2):
                h = 2 * hp + hh
                d = 2 * (h % 2) + (h // 2) // 4
                hg = (h // 2) % 4
                pb = 32 * d
                pt = apool.tile([128, KT, T], fp8, tag="pt", name="pt", bufs=2)
                pv = psatt.tile([128, T], f32, tag="pv", name="pv", bufs=2)
                for g in range(KT // 2):
                    sc = psatt.tile([128, 2, T], f32, tag="sc", name="sc", bufs=2)
                    for j in range(2):
                        kt = 2 * g + j
                        if kt < KTC:
                            lhs = kc_sb[r][pb:pb + 32, :, hg,
                                           kt * 128:(kt + 1) * 128]
                        else:
                            lhs = kD[r][pb:pb + 32, :, hg,
                                        (kt - KTC) * 128:(kt - KTC + 1) * 128]
                        nc.tensor.matmul(sc[:, j, :], lhs,
                                         qD[r][pb:pb + 32, :, hg, :],
                                         start=True, stop=True, perf_mode=DRm,
                                         tile_position=(pb, 0))
                    nc.scalar.activation(
                        pt[:, 2 * g:2 * g + 2, :].rearrange("p a b -> p (a b)"),
                        sc[:].rearrange("p a b -> p (a b)"), AF.Exp, scale=SCALE)
                    half = 0 if g < KTC // 2 else 1
                    koff = 0 if g < KTC // 2 else KTC
                    nc.tensor.matmul(
                        pv[0:VP, :],
                        vkv[r][:, half, h, 2 * g - koff:2 * g - koff + 2, :],
                        pt[:, 2 * g:2 * g + 2, :],
                        start=(g == 0), stop=(g == KT // 2 - 1), perf_mode=DRm)
                rd = apool.tile([1, T], f32, tag="rd", name="rd", bufs=2)
                nc.vector.reciprocal(rd[:], pv[HD:HD + 1, :])
                bcs = apool.tile([HD, T], f32, tag="bcs", name="bcs", bufs=2)
                nc.gpsimd.partition_broadcast(bcs[:], rd[:])
                nc.vector.tensor_mul(oT[r][64 * hh:64 * hh + 64, hp, :],
                                     pv[0:HD, :], bcs[:])

        def emit_proj(co, r, wpool, mpool):
            wt = wpool.tile([128, CT, 128], fp8, tag="wp", name="wp", bufs=3)
            nc.sync.dma_start(wt[:], wp.ap()[co])
            ps = mpool.tile([128, T], f32, tag="misc", name="misc", bufs=2)
            for c in range(CT // 2):
                nc.tensor.matmul(ps[:], wt[:, 2 * c:2 * c + 2, :],
                                 oT[r][:, 2 * c:2 * c + 2, :],
                                 start=(c == 0),
                                 stop=(not with_bias and c == CT // 2 - 1),
                                 perf_mode=DRm)
            if with_bias:
                nc.tensor.matmul(ps[:], wxp_sb[0:1, co, :, :], xtr_p[0:1, :, :],
                                 start=False, stop=True, perf_mode=DRm)
            nc.vector.scalar_tensor_tensor(
                x2s[r][:, co, :], ps[:], 1.0 / (OS * WS),
                xs[r][:, co, :].bitcast(f32), OP.mult, OP.add)

        def emit_fc1_pair(htp, rows, wpool, mpool):
            wt = wpool.tile([128, 2, 2, CT, 128], fp8, tag="w1", name="w1", bufs=3)
            nc.sync.dma_start(wt[:], w1.ap()[htp])
            for r in rows:
                for j in range(2):
                    ht = 2 * htp + j
                    ps = mpool.tile([128, T], f32, tag="misc", name="misc", bufs=2)
                    for pi, (pw, ph) in enumerate(((0, 0), (0, 1), (1, 0))):
                        for c in range(CT // 2):
                            nc.tensor.matmul(ps[:],
                                             wt[:, j, pw, 2 * c:2 * c + 2, :],
                                             h2[r][:, ph, 2 * c:2 * c + 2, :],
                                             start=(pi == 0 and c == 0),
                                             stop=(not with_bias and pi == 2
                                                   and c == CT // 2 - 1),
                                             perf_mode=DRm)
                    if with_bias:
                        nc.tensor.matmul(ps[:], wx1_sb[0:1, ht, :, :],
                                         xtr_1[0:1, :, :],
                                         start=False, stop=True, perf_mode=DRm)
                    nc.scalar.activation(gs[r][:, ht, :], ps[:], AF.Gelu,
                                         scale=1.0 / WS)

        def emit_fc2(co, r, wt, f2pool, opool):
            ps = f2pool.tile([128, T], f32, tag="fc2", name="fc2", bufs=3)
            for pw in range(W2P):
                for tp in range(NHT // 2):
                    nc.tensor.matmul(ps[:], wt[:, pw, 2 * tp:2 * tp + 2, :],
                                     gs[r][:, 2 * tp:2 * tp + 2, :],
                                     start=(pw == 0 and tp == 0),
                                     stop=(not with_bias and pw == W2P - 1
                                           and tp == NHT // 2 - 1),
                                     perf_mode=DRm)
            if with_bias:
                nc.tensor.matmul(ps[:], wx2_sb[0:1, co, :, :], xtr_1[0:1, :, :],
                                 start=False, stop=True, perf_mode=DRm)
            ot = opool.tile([128, T], f32, tag="ot", name="ot", bufs=3)
            nc.vector.scalar_tensor_tensor(
                ot[:], ps[:], 1.0 / WS, x2s[r][:, co, :], OP.mult, OP.add)
            nc.sync.dma_start(outT.ap()[r, :, co, :], ot[:])

        # ================= phase 0: loads + LN1 =================
        cm_psln, psln = open_pool("ps_ln", space="PSUM")
        for r in range(RPC):
            nc.sync.dma_start(xs[r][:, 0:CT // 2, :], xT.ap()[r, :, 0:CT // 2, :])
            nc.sync.dma_start(xs[r][:, CT // 2:, :], xT.ap()[r, :, CT // 2:, :])
            nc.sync.dma_start(
                kc_sb[r][:].rearrange("p a b c -> p (a b c)"), kcD.ap()[r])
            nc.sync.dma_start(
                vkv[r][:, 0, :, :, :].rearrange("p h k d -> p (h k d)"),
                vcC.ap()[r])
            nc.gpsimd.memset(vkv[r][:, 1, :, :, HD:HD + 1], 1.0)
            nc.gpsimd.memset(vkv[r][:, 1, :, :, HD + 1:VP], 0.0)
        for r in range(RPC):
            layernorm(xs[r], h1[r], psln, "l1", ones_sb[:, 0:1],
                      lambda ap: ap.bitcast(f32))
        close_pool(cm_psln)

        # ================= qkv r0 + swizzle r0 =================
        cm_wA, wA = open_pool("p_wA")
        cm_psmm, psmm = open_pool("ps_mm", space="PSUM")
        cm_st0, p_st0 = open_pool("p_st0")
        q0 = p_st0.tile([128, CT, T], fp8, tag="q0", name="q0")
        k0 = p_st0.tile([128, CT, T], fp8, tag="k0", name="k0")
        for f in range(4):
            emit_qk_pair(f, 0, wA, psmm, q0, k0)
        emit_v_chunk(0, 0, wA, psmm)
        emit_v_chunk(1, 0, wA, psmm)
        emit_swizzle(0, q0, k0, 0)
        for f in range(4, 8):
            emit_qk_pair(f, 0, wA, psmm, q0, k0)
        emit_v_chunk(2, 0, wA, psmm)
        emit_v_chunk(3, 0, wA, psmm)
        emit_swizzle(0, q0, k0, 1)
        close_pool(cm_st0)

        # ================= phase A: attention r0 || qkv r1 =================
        cm_st1, p_st1 = open_pool("p_st1")
        q1 = p_st1.tile([128, CT, T], fp8, tag="q1", name="q1")
        k1 = p_st1.tile([128, CT, T], fp8, tag="k1", name="k1")
        for hp in range(HPAIR):
            emit_attention(hp, 0)
        for f in range(8):
            emit_qk_pair(f, 1, wA, psmm, q1, k1)
            if f % 2 == 1:
                emit_v_chunk(f // 2, 1, wA, psmm)
            if f == 3:
                emit_swizzle(1, q1, k1, 0)
        emit_swizzle(1, q1, k1, 1)
        close_pool(cm_st1)
        close_pool(cm_psmm)
        close_pool(cm_wA)
        close_pool(cm_h1)

        # ====== phase B: attention r1 (priority) || proj/LN2/FC1 r0 ======
        cm_psB, psB = open_pool("ps_B", space="PSUM")
        cm_wB, wB = open_pool("p_wB")
        for hp in range(HPAIR):
            emit_attention(hp, 1)
        for co in range(CT):
            emit_proj(co, 0, wB, psB)
        layernorm(x2s[0], h2[0][:, 0], psB, "l2a", ones_bf[:],
                  lambda ap: ap, dst_lo=h2[0][:, 1])
        for htp in range(NHT // 2):
            emit_fc1_pair(htp, [0], wB, psB)
        close_pool(cm_wB)
        close_pool(cm_psB)
        close_pool(cm_psatt)
        close_pool(cm_att)
        close_pool(cm_kv)

        # ================= tail =================
        with tc.tile_pool(name="ps_T", space="PSUM", bufs=1) as psT, \
             tc.tile_pool(name="p_wT", bufs=1) as wT, \
             tc.tile_pool(name="p_osb", bufs=1) as osb:
            for co in range(CT):
                emit_proj(co, 1, wT, psT)
            layernorm(x2s[1], h2[1][:, 0], psT, "l2b", ones_bf[:],
                      lambda ap: ap, dst_lo=h2[1][:, 1])
            for co in range(CT):
                w2t = wT.tile([128, W2P, NHT, 128], fp8, tag="w2", name="w2", bufs=2)
                nc.sync.dma_start(w2t[:], w2.ap()[co])
                emit_fc2(co, 0, w2t, psT, osb)
            for htp in range(NHT // 2):
                emit_fc1_pair(htp, [1], wT, psT)
            for co in range(CT):
                w2t = wT.tile([128, W2P, NHT, 128], fp8, tag="w2", name="w2", bufs=2)
                nc.sync.dma_start(w2t[:], w2.ap()[co])
                emit_fc2(co, 1, w2t, psT, osb)
        close_pool(cm_h2)
        close_pool(cm_g)
        close_pool(cm_x2)
        close_pool(cm_oT)
        close_pool(cm_xs)
        close_pool(cm_work)

    nc.compile()
    return nc


class _Runner:
    """Hold the compiled PJRT executable (mirrors bass2jax.run_bass_via_pjrt)."""

    def __init__(self, nc, n_cores):
        import jax
        from jax.sharding import Mesh, PartitionSpec
        from jax.experimental.shard_map import shard_map
        import concourse.mybir as mybir
        from concourse.bass2jax import (
            install_neuronx_cc_hook, partition_id_tensor, _bass_exec_p)

        install_neuronx_cc_hook()
        self.jax = jax
        self.n_cores = n_cores
        partition_name = nc.partition_id_tensor.name if nc.partition_id_tensor else None
        in_names, out_names, out_avals, zero_outs = [], [], [], []
        for alloc in nc.m.functions[0].allocations:
            if not isinstance(alloc, mybir.MemoryLocationSet):
                continue
            name = alloc.memorylocations[0].name
            if alloc.kind == "ExternalInput":
                if name != partition_name:
                    in_names.append(name)
            elif alloc.kind == "ExternalOutput":
                shape = tuple(alloc.tensor_shape)
                dtype = mybir.dt.np(alloc.dtype)
                out_names.append(name)
                out_avals.append(jax.core.ShapedArray(shape, dtype))
                zero_outs.append(np.zeros(shape, dtype))
        self.in_names, self.out_names = in_names, out_names
        self.out_avals, self.zero_outs = out_avals, zero_outs
        self.n_params = len(in_names)
        all_names = in_names + out_names
        if partition_name is not None:
            all_names.append(partition_name)

        def _body(*args):
            operands = list(args)
            if partition_name is not None:
                operands.append(partition_id_tensor())
            return tuple(
                _bass_exec_p.bind(
                    *operands,
                    out_avals=tuple(out_avals),
                    in_names=tuple(all_names),
                    out_names=tuple(out_names),
                    lowering_input_output_aliases=(),
                    sim_require_finite=True,
                    sim_require_nnan=True,
                    nc=nc,
                ))

        devices = jax.devices()[:n_cores]
        assert len(devices) == n_cores, f"need {n_cores} cores, have {len(jax.devices())}"
        mesh = Mesh(np.asarray(devices), ("core",))
        n_outs = len(out_names)
        self._fn = jax.jit(
            shard_map(_body, mesh=mesh,
                      in_specs=(PartitionSpec("core"),) * (self.n_params + n_outs),
                      out_specs=(PartitionSpec("core"),) * n_outs,
                      check_rep=False),
            keep_unused=True)

    def prepare(self, in_maps):
        np_ = np
        per_core = [[np_.asarray(m[n]) for n in self.in_names] for m in in_maps]
        concat_in = [
            np_.concatenate([per_core[c][i] for c in range(self.n_cores)], axis=0)
            for i in range(self.n_params)]
        concat_zeros = [
            np_.zeros((self.n_cores * z.shape[0], *z.shape[1:]), z.dtype)
            for z in self.zero_outs]
        return self.jax.device_put(concat_in + concat_zeros)

    def run(self, prepared):
        out = self._fn(*prepared)
        self.jax.block_until_ready(out)
        return out

    def results(self, out_arrs):
        return [
            {name: np.asarray(out_arrs[i]).reshape(
                self.n_cores, *self.out_avals[i].shape)[c]
             for i, name in enumerate(self.out_names)}
            for c in range(self.n_cores)]


def _get_runner(with_bias):
    if "runner" not in _state:
        nc = _build_module(with_bias=with_bias)
        _state["nc"] = nc
        _state["runner"] = _Runner(nc, NCORES)
    return _state["runner"]


def _prepare_in_maps(x, cache_k, cache_v, update_mask, qkv_w, qkv_b, proj_w,
                     proj_b, n1_g, n1_b, n2_g, n2_b, fc1_w, fc1_b, fc2_w, fc2_b):
    f32 = np.float32
    x = np.asarray(x, f32)
    cache_k = np.asarray(cache_k, f32)
    cache_v = np.asarray(cache_v, f32)
    update_mask = np.asarray(update_mask, bool)
    qkv_w = np.asarray(qkv_w, f32)
    qkv_b = np.asarray(qkv_b, f32)
    proj_w = np.asarray(proj_w, f32)
    proj_b = np.asarray(proj_b, f32)
    n1_g = np.asarray(n1_g, f32)
    n1_b = np.asarray(n1_b, f32)
    n2_g = np.asarray(n2_g, f32)
    n2_b = np.asarray(n2_b, f32)
    fc1_w = np.asarray(fc1_w, f32)
    fc1_b = np.asarray(fc1_b, f32)
    fc2_w = np.asarray(fc2_w, f32)
    fc2_b = np.asarray(fc2_b, f32)

    xT = np.ascontiguousarray(
        x.transpose(0, 2, 1).reshape(B, CT, 128, T).transpose(0, 2, 1, 3))

    kcD = np.empty((B, 128, 2, 4, NKC), fp8np)
    vcC = np.empty((B, 128, H, KTC, VP), fp8np)
    for b in range(B):
        keep = ~update_mask[b]
        kc = cache_k[b][:, keep, :]          # [H, NKC, HD]
        vc = cache_v[b][:, keep, :]
        kt_ = kc.transpose(0, 2, 1)          # [H, HD, NKC]
        for h in range(H):
            d_ = 2 * (h % 2) + (h // 2) // 4
            hg_ = (h // 2) % 4
            for half in range(2):
                kcD[b, 32 * d_:32 * d_ + 32, half, hg_, :] = (
                    kt_[h, 32 * half:32 * half + 32, :].astype(fp8np))
        vv = (vc.transpose(1, 0, 2).reshape(KTC, 128, H, HD)
              .transpose(1, 2, 0, 3))        # [128, H, KTC, HD]
        pad = np.zeros((128, H, KTC, VP - HD), f32)
        pad[:, :, :, 0] = 1.0
        vcC[b] = np.concatenate([vv * OS, pad], axis=3).astype(fp8np)

    def wtile(w, nf, cols):
        ci = w.shape[0]
        return np.ascontiguousarray(
            (WS * w).reshape(ci // 128, 128, nf, cols)
            .transpose(2, 1, 0, 3)).astype(fp8np)

    def wtile_hl(w, nf, cols):
        ws = WS * w
        hi = ws.astype(fp8np)
        lo = (ws - hi.astype(np.float32)).astype(fp8np)
        ci = w.shape[0]

        def t(a):
            return (a.reshape(ci // 128, 128, nf, cols).transpose(2, 1, 0, 3))
        return np.ascontiguousarray(
            np.stack([t(hi), t(lo)], axis=2)).astype(fp8np)

    wqkv_eff = n1_g[:, None] * qkv_w
    bias_qkv = n1_b @ qkv_w + qkv_b
    wqk16 = wtile(wqkv_eff[:, :2048], 16, 128)       # [16,128,CT,128]
    wqk_t = np.empty((8, 128, 2, CT, 128), fp8np)
    for f in range(8):
        wqk_t[f, :, 0] = wqk16[f]
        wqk_t[f, :, 1] = wqk16[8 + f]
    wv_t = wtile(wqkv_eff[:, 2048:], 4, 256)
    bqk_t = np.ascontiguousarray(bias_qkv[:2048].reshape(16, 128).T).astype(f32)
    vbias_t = (OS * bias_qkv[2048:]).astype(f32)

    wp_t = wtile(proj_w, CT, 128)
    wxp_t = np.zeros((1, CT, 2, 128), fp8np)
    wxp_t[0, :, 0, :] = (WS * proj_b).reshape(CT, 128).astype(fp8np)

    w1_eff = n2_g[:, None] * fc1_w
    bias_fc1 = n2_b @ fc1_w + fc1_b
    w1_hl = wtile_hl(w1_eff, NHT, 128)               # [32,128,2,CT,128]
    w1_t = np.ascontiguousarray(
        w1_hl.reshape(NHT // 2, 2, 128, 2, CT, 128)
        .transpose(0, 2, 1, 3, 4, 5))                # [16,128,2,2,CT,128]
    wx1_t = np.zeros((1, NHT, 2, 128), fp8np)
    wx1_t[0, :, 0, :] = (WS * bias_fc1).reshape(NHT, 128).astype(fp8np)

    w2_t = np.ascontiguousarray(
        wtile_hl(fc2_w, CT, 128)[:, :, :W2P])        # [CT,128,W2P,NHT,128]
    wx2_t = np.zeros((1, CT, 2, 128), fp8np)
    wx2_t[0, :, 0, :] = (WS * fc2_b).reshape(CT, 128).astype(fp8np)

    with_bias = bool(np.any(wxp_t) or np.any(wx1_t) or np.any(wx2_t))
    shared = dict(
        wqk=wqk_t, wv=wv_t, wp=wp_t, w1=w1_t, w2=w2_t,
        bqk=bqk_t, vbias=vbias_t,
        ones=np.ones((128, 1), f32),
    )
    if with_bias:
        shared.update(wxp=wxp_t, wx1=wx1_t, wx2=wx2_t)
    in_maps = []
    for c in range(NCORES):
        s = slice(c * RPC, (c + 1) * RPC)
        in_maps.append(dict(
            shared, xT=xT[s], kcD=kcD[s],
            vcC=vcC[s].reshape(RPC, 128, H * KTC * VP)))
    return in_maps, with_bias


def kernel(**inputs) -> np.ndarray:
    in_maps, with_bias = _prepare_in_maps(**inputs)
    runner = _get_runner(with_bias)
    prepared = runner.prepare(in_maps)
    out = runner.run(prepared)
    res = runner.results(out)
    full = np.empty((B, NP, C), np.float32)
    for c in range(NCORES):
        for r in range(RPC):
            full[c * RPC + r] = res[c]["outT"][r].transpose(2, 1, 0).reshape(T, C)
    return full


# revision 31
# speedup vs baseline: 1.7758x; 1.0055x over previous
"""Fused decoder block (LN->QKV->cache-merge attention->proj->LN->MLP) on 8
Trainium2 NeuronCores, data-parallel over the batch (2 rows/core).

v3: row-pipelined schedule. The two batch rows per core are independent, so
the ACT-bound attention of row 1 overlaps the PE-bound proj/LN2/FC1 of row 0.

Key ideas (cumulative):
- host-side cache compaction via update_mask: attention over [512 surviving
  cache keys ++ 512 new keys] = 1024 keys, no mask bias (softmax is
  permutation invariant).
- fp8e4 DoubleRow matmuls everywhere (QKV, scores, PV, proj, FC1, FC2);
  weights host-quantized at x64 scale; hi+lo residual passes for W1/W2/h2
  keep the MLP path accurate (rel err ~1.3e-2 < 2e-2).
- q/k swizzled into [32, 2, head, T] pair layout by SBUF->SBUF DMAs so the
  64-dim head contraction runs as DoubleRow (half cost).
- LN standardization only (gains/biases folded into weights host-side);
  sum-sq stats via fp8 DR on squared tiles; rstd/-mu broadcast across
  partitions by GPSIMD partition_broadcast (no PE, no PSUM).
- linear biases as extra fp8 contraction rows; softmax denominator from a
  ones column in V, reciprocal broadcast on Pool.
- x2 residual kept in bf16 to fit SBUF; PSUM pools sized to exactly 8 banks
  per phase.
"""

import numpy as np
import ml_dtypes

B, NP, N, C, H = 16, 512, 1024, 1024, 16
HD = C // H            # 64
HID = 4 * C            # 4096
EPS = 1e-5
NCORES = 8
RPC = B // NCORES      # batch rows per core
T = NP                 # queries per row
CT = C // 128          # feature tiles
NKC = N - NP           # surviving cache keys (512)
KTC = NKC // 128       # cache key tiles (4)
KTN = T // 128         # new key tiles (4)
KT = KTC + KTN         # total key tiles (8)
HPAIR = H // 2
NHT = HID // 128       # fc1 output chunks (32)
SCALE = HD ** -0.5
WS = 64.0              # weight quantization scale
OS = 16.0              # v / attention-output scale
VP = 80                # padded v row (64 d + 1 ones + 15 pad)
W2P = 1                # fc2 weight passes (1 = hi only, 2 = hi+lo residual)

_state = {}
fp8np = ml_dtypes.float8_e4m3


def _build_module(with_bias=True):
    import concourse.tile as tile
    from concourse import bacc, mybir

    f32 = mybir.dt.float32
    f32r = mybir.dt.float32r
    bf16 = mybir.dt.bfloat16
    fp8 = mybir.dt.float8e4
    AF = mybir.ActivationFunctionType
    OP = mybir.AluOpType
    DRm = mybir.MatmulPerfMode.DoubleRow

    nc = bacc.Bacc("TRN2", target_bir_lowering=False, debug=False)

    xT = nc.dram_tensor("xT", [RPC, 128, CT, T], f32r, kind="ExternalInput")
    kcD = nc.dram_tensor("kcD", [RPC, 128, 2, 4, NKC], fp8, kind="ExternalInput")
    vcC = nc.dram_tensor("vcC", [RPC, 128, H * KTC * VP], fp8, kind="ExternalInput")
    wqk = nc.dram_tensor("wqk", [8, 128, 2, CT, 128], fp8, kind="ExternalInput")
    wv = nc.dram_tensor("wv", [4, 128, CT, 256], fp8, kind="ExternalInput")
    wp = nc.dram_tensor("wp", [CT, 128, CT, 128], fp8, kind="ExternalInput")
    w1 = nc.dram_tensor("w1", [NHT // 2, 128, 2, 2, CT, 128], fp8,
                        kind="ExternalInput")
    w2 = nc.dram_tensor("w2", [CT, 128, W2P, NHT, 128], fp8, kind="ExternalInput")
    if with_bias:
        wxp = nc.dram_tensor("wxp", [1, CT, 2, 128], fp8, kind="ExternalInput")
        wx1 = nc.dram_tensor("wx1", [1, NHT, 2, 128], fp8, kind="ExternalInput")
        wx2 = nc.dram_tensor("wx2", [1, CT, 2, 128], fp8, kind="ExternalInput")
    bqk = nc.dram_tensor("bqk", [128, 16], f32, kind="ExternalInput")
    vbias = nc.dram_tensor("vbias", [C], f32, kind="ExternalInput")
    ones = nc.dram_tensor("ones", [128, 1], f32r, kind="ExternalInput")
    outT = nc.dram_tensor("outT", [RPC, 128, CT, T], f32, kind="ExternalOutput")

    from contextlib import ExitStack
    with nc.allow_low_precision(reason="deliberate fp8/f32r staging; PSUM accumulation fp32"), \
         tile.TileContext(nc, pool_alloc_mode="queue") as tc, ExitStack() as es:
        # ---------- constants ----------
        consts = es.enter_context(tc.tile_pool(name="consts", bufs=1))
        ones_sb = consts.tile([128, 1], f32r)
        nc.sync.dma_start(ones_sb[:], ones.ap())
        ones_bf = consts.tile([128, 1], bf16)
        nc.vector.memset(ones_bf[:], 1.0)
        ones8 = consts.tile([128, 2, 16], fp8)
        nc.vector.memset(ones8[:], 1.0)
        bqk_sb = consts.tile([128, 16], f32)
        nc.sync.dma_start(bqk_sb[:], bqk.ap())
        if with_bias:
            vb_sb = consts.tile([128, C], bf16)
            nc.gpsimd.dma_start(vb_sb[:], vbias.ap()[None].to_broadcast((128, C)))
        if with_bias:
            wxp_sb = consts.tile([1, CT, 2, 128], fp8)
            nc.sync.dma_start(wxp_sb[:], wxp.ap())
            wx1_sb = consts.tile([1, NHT, 2, 128], fp8)
            nc.sync.dma_start(wx1_sb[:], wx1.ap())
            wx2_sb = consts.tile([1, CT, 2, 128], fp8)
            nc.sync.dma_start(wx2_sb[:], wx2.ap())
            xtr_p = consts.tile([1, 2, 512], fp8)
            nc.vector.memset(xtr_p[0:1, 0, :], OS)
            nc.vector.memset(xtr_p[0:1, 1, :], 0.0)
            xtr_1 = consts.tile([1, 2, 512], fp8)
            nc.vector.memset(xtr_1[0:1, 0, :], 1.0)
            nc.vector.memset(xtr_1[0:1, 1, :], 0.0)
        eps_sb = consts.tile([1, 1], f32)
        nc.vector.memset(eps_sb[:], EPS)

        def open_pool(nm, space=None):
            kw = dict(space=space) if space else {}
            cm = tc.tile_pool(name=nm, bufs=1, **kw)
            return cm, cm.__enter__()

        def close_pool(cm):
            cm.__exit__(None, None, None)

        # ---------- long-lived pools (ring-stack order) ----------
        cm_work, p_work = open_pool("p_work")
        cm_xs, p_xs = open_pool("p_xs")
        xs = [p_xs.tile([128, CT, T], f32r, tag=f"xs{r}", name=f"xs{r}")
              for r in range(RPC)]
        cm_oT, p_oT = open_pool("p_oT")
        oT = [p_oT.tile([128, CT, T], fp8, tag=f"oT{r}", name=f"oT{r}")
              for r in range(RPC)]
        cm_x2, p_x2 = open_pool("p_x2")
        x2s = [p_x2.tile([128, CT, T], bf16, tag=f"x2{r}", name=f"x2{r}")
               for r in range(RPC)]
        cm_g, p_g = open_pool("p_g")
        gs = [p_g.tile([128, NHT, T], fp8, tag=f"g{r}", name=f"g{r}")
              for r in range(RPC)]
        cm_h2, p_h2 = open_pool("p_h2")
        h2 = [p_h2.tile([128, 2, CT, T], fp8, tag=f"h2{r}", name=f"h2{r}")
              for r in range(RPC)]
        cm_kv, p_kv = open_pool("p_kv")
        kc_sb = [p_kv.tile([128, 2, 4, NKC], fp8, tag=f"kc{r}", name=f"kc{r}")
                 for r in range(RPC)]
        vkv = [p_kv.tile([128, 2, H, KTC, VP], fp8, tag=f"vkv{r}", name=f"vkv{r}")
               for r in range(RPC)]
        qD = [p_kv.tile([128, 2, 4, T], fp8, tag=f"qD{r}", name=f"qD{r}")
              for r in range(RPC)]
        kD = [p_kv.tile([128, 2, 4, T], fp8, tag=f"kD{r}", name=f"kD{r}")
              for r in range(RPC)]
        cm_att, apool = open_pool("p_att")
        cm_psatt, psatt = open_pool("ps_att", space="PSUM")
        cm_h1, p_h1 = open_pool("p_h1")
        h1 = [p_h1.tile([128, CT, T], fp8, tag=f"h1{r}", name=f"h1{r}")
              for r in range(RPC)]

        # ================= layernorm =================
        def layernorm(src, dst, stats_pool, st_tag, lhs_ones, src_cast,
                      dst_lo=None):
            s_t = stats_pool.tile([128, T], f32, tag="misc",
                                  name=f"{st_tag}s", bufs=2)
            s_ps = s_t[0:1, :]
            for ct in range(CT):
                nc.tensor.matmul(s_ps, lhs_ones, src[:, ct, :],
                                 start=(ct == 0), stop=(ct == CT - 1))
            ss_t = stats_pool.tile([128, T], f32, tag="misc",
                                   name=f"{st_tag}ss", bufs=2)
            ss_ps = ss_t[0:1, :]
            for c in range(CT // 2):
                sqs = p_work.tile([128, 2, T], fp8, tag="sqs", name="sqs", bufs=1)
                for j in range(2):
                    eng = nc.gpsimd if j else nc.vector
                    eng.tensor_mul(sqs[:, j, :], src_cast(src[:, 2 * c + j, :]),
                                   src_cast(src[:, 2 * c + j, :]))
                nc.tensor.matmul(ss_ps, ones8[:, :, 0:1], sqs[:],
                                 start=(c == 0), stop=(c == CT // 2 - 1),
                                 perf_mode=DRm)
            st = p_work.tile([97, T], f32, tag="st", name="st", bufs=1)
            negmu, msq, var, stdv = (st[0:1, :], st[32:33, :], st[64:65, :],
                                     st[96:97, :])
            nc.vector.tensor_scalar(negmu, s_ps, -1.0 / C, None, OP.mult)
            nc.vector.tensor_mul(msq, negmu, negmu)
            nc.vector.scalar_tensor_tensor(var, ss_ps, 1.0 / C, msq,
                                           OP.mult, OP.subtract)
            nc.scalar.activation(stdv, var, AF.Sqrt, bias=eps_sb[:])
            rstd = p_work.tile([1, T], f32, tag="rstd", name="rstd", bufs=2)
            nc.vector.reciprocal(rstd[:], stdv)
            A_sb = p_work.tile([128, T], f32, tag="Asb", name="Asb", bufs=1)
            nc.gpsimd.partition_broadcast(A_sb[:], rstd[:])
            M_sb = p_work.tile([128, T], f32, tag="Msb", name="Msb", bufs=1)
            nc.gpsimd.partition_broadcast(M_sb[:], negmu)
            for ct in range(CT):
                tmp = p_work.tile([128, T], f32, tag="tmp", name="tmp", bufs=2)
                e1, e2 = ((nc.gpsimd, nc.vector) if ct % 3 == 2
                          else (nc.vector, nc.gpsimd)
                          if ct % 3 == 1 else (nc.vector, nc.vector))
                e1.tensor_add(tmp[:], src_cast(src[:, ct, :]), M_sb[:])
                if dst_lo is None:
                    e2.tensor_mul(dst[:, ct, :], tmp[:], A_sb[:])
                else:
                    e2.tensor_mul(tmp[:], tmp[:], A_sb[:])
                    e1.tensor_copy(dst[:, ct, :], tmp[:])
                    nc.vector.scalar_tensor_tensor(
                        dst_lo[:, ct, :], dst[:, ct, :], -1.0, tmp[:],
                        OP.mult, OP.add)

        # ================= emitters =================
        def emit_qk_pair(f, r, wpool, mmps, qsb, ksb):
            wt = wpool.tile([128, 2, CT, 128], fp8, tag="wqk", name="wqk", bufs=3)
            nc.sync.dma_start(wt[:], wqk.ap()[f])
            for which in range(2):
                ps = mmps.tile([128, T], f32, tag="mm", name="mm", bufs=2)
                for c in range(CT // 2):
                    nc.tensor.matmul(ps[:], wt[:, which, 2 * c:2 * c + 2, :],
                                     h1[r][:, 2 * c:2 * c + 2, :],
                                     start=(c == 0), stop=(c == CT // 2 - 1),
                                     perf_mode=DRm)
                dst = (qsb if which == 0 else ksb)[:, f, :]
                nc.vector.tensor_scalar(dst, ps[:], 1.0 / WS,
                                        bqk_sb[:, 8 * which + f:8 * which + f + 1],
                                        OP.mult, OP.add)

        def emit_v_chunk(ch, r, wpool, mmps):
            wvt = wpool.tile([128, CT, 256], fp8, tag="wv", name="wv", bufs=2)
            nc.sync.dma_start(wvt[:], wv.ap()[ch])
            for tt in range(KTN):
                psf = mmps.tile([128, T], f32, tag="mm", name="mm", bufs=2)
                ps = psf[:, 0:256]
                for c in range(CT // 2):
                    nc.tensor.matmul(
                        ps, h1[r][:, 2 * c:2 * c + 2, tt * 128:(tt + 1) * 128],
                        wvt[:, 2 * c:2 * c + 2, :],
                        start=(c == 0), stop=(c == CT // 2 - 1), perf_mode=DRm)
                if with_bias:
                    nc.vector.scalar_tensor_tensor(
                        vkv[r][:, 1, 4 * ch:4 * ch + 4, tt, 0:HD],
                        ps.rearrange("p (h d) -> p h d", h=4), OS / WS,
                        vb_sb[:, ch * 256:(ch + 1) * 256]
                        .rearrange("p (h d) -> p h d", h=4),
                        OP.mult, OP.add)
                else:
                    nc.vector.tensor_scalar(
                        vkv[r][:, 1, 4 * ch:4 * ch + 4, tt, 0:HD],
                        ps.rearrange("p (h d) -> p h d", h=4), OS / WS,
                        None, OP.mult)

        def emit_swizzle(r, qsb, ksb, fq):
            # head h=2f+par -> block d=2*par+f//4, hg=f%4
            for src_sb, dst in ((qsb, qD[r]), (ksb, kD[r])):
                for par in range(2):
                    for half in range(2):
                        nc.sync.dma_start(
                            dst[32 * (2 * par + fq):32 * (2 * par + fq) + 32,
                                half, :, :],
                            src_sb[64 * par + 32 * half:
                                   64 * par + 32 * half + 32,
                                   4 * fq:4 * fq + 4, :])

        def emit_attention(hp, r):
            for hh in range(2):
                h = 2 * hp + hh
                d = 2 * (h % 2) + (h // 2) // 4
                hg = (h // 2) % 4
                pb = 32 * d
                pt = apool.tile([128, KT, T], fp8, tag="pt", name="pt", bufs=2)
                pv = psatt.tile([128, T], f32, tag="pv", name="pv", bufs=2)
                for g in range(KT // 2):
                    sc = psatt.tile([128, 2, T], f32, tag="sc", name="sc", bufs=2)
                    for j in range(2):
                        kt = 2 * g + j
                        if kt < KTC:
                            lhs = kc_sb[r][pb:pb + 32, :, hg,
                                           kt * 128:(kt + 1) * 128]
                        else:
                            lhs = kD[r][pb:pb + 32, :, hg,
                                        (kt - KTC) * 128:(kt - KTC + 1) * 128]
                        nc.tensor.matmul(sc[:, j, :], lhs,
                                         qD[r][pb:pb + 32, :, hg, :],
                                         start=True, stop=True, perf_mode=DRm,
                                         tile_position=(pb, 0))
                    nc.scalar.activation(
                        pt[:, 2 * g:2 * g + 2, :].rearrange("p a b -> p (a b)"),
                        sc[:].rearrange("p a b -> p (a b)"), AF.Exp, scale=SCALE)
                    half = 0 if g < KTC // 2 else 1
                    koff = 0 if g < KTC // 2 else KTC
                    nc.tensor.matmul(
                        pv[0:VP, :],
                        vkv[r][:, half, h, 2 * g - koff:2 * g - koff + 2, :],
                        pt[:, 2 * g:2 * g + 2, :],
                        start=(g == 0), stop=(g == KT // 2 - 1), perf_mode=DRm)
                rd = apool.tile([1, T], f32, tag="rd", name="rd", bufs=2)
                nc.vector.reciprocal(rd[:], pv[HD:HD + 1, :])
                bcs = apool.tile([HD, T], f32, tag="bcs", name="bcs", bufs=2)
                nc.gpsimd.partition_broadcast(bcs[:], rd[:])
                nc.vector.tensor_mul(oT[r][64 * hh:64 * hh + 64, hp, :],
                                     pv[0:HD, :], bcs[:])

        def emit_proj(co, r, wpool, mpool):
            wt = wpool.tile([128, CT, 128], fp8, tag="wp", name="wp", bufs=3)
            nc.sync.dma_start(wt[:], wp.ap()[co])
            ps = mpool.tile([128, T], f32, tag="misc", name="misc", bufs=2)
            for c in range(CT // 2):
                nc.tensor.matmul(ps[:], wt[:, 2 * c:2 * c + 2, :],
                                 oT[r][:, 2 * c:2 * c + 2, :],
                                 start=(c == 0),
                                 stop=(not with_bias and c == CT // 2 - 1),
                                 perf_mode=DRm)
            if with_bias:
                nc.tensor.matmul(ps[:], wxp_sb[0:1, co, :, :], xtr_p[0:1, :, :],
                                 start=False, stop=True, perf_mode=DRm)
            nc.vector.scalar_tensor_tensor(
                x2s[r][:, co, :], ps[:], 1.0 / (OS * WS),
                xs[r][:, co, :].bitcast(f32), OP.mult, OP.add)

        def emit_fc1_pair(htp, rows, wpool, mpool):
            wt = wpool.tile([128, 2, 2, CT, 128], fp8, tag="w1", name="w1", bufs=3)
            nc.sync.dma_start(wt[:], w1.ap()[htp])
            for r in rows:
                for j in range(2):
                    ht = 2 * htp + j
                    ps = mpool.tile([128, T], f32, tag="misc", name="misc", bufs=2)
                    for pi, (pw, ph) in enumerate(((0, 0), (0, 1), (1, 0))):
                        for c in range(CT // 2):
                            nc.tensor.matmul(ps[:],
                                             wt[:, j, pw, 2 * c:2 * c + 2, :],
                                             h2[r][:, ph, 2 * c:2 * c + 2, :],
                                             start=(pi == 0 and c == 0),
                                             stop=(not with_bias and pi == 2
                                                   and c == CT // 2 - 1),
                                             perf_mode=DRm)
                    if with_bias:
                        nc.tensor.matmul(ps[:], wx1_sb[0:1, ht, :, :],
                                         xtr_1[0:1, :, :],
                                         start=False, stop=True, perf_mode=DRm)
                    nc.scalar.activation(gs[r][:, ht, :], ps[:], AF.Gelu,
                                         scale=1.0 / WS)

        def emit_fc2(co, r, wt, f2pool, opool):
            ps = f2pool.tile([128, T], f32, tag="fc2", name="fc2", bufs=3)
            for pw in range(W2P):
                for tp in range(NHT // 2):
                    nc.tensor.matmul(ps[:], wt[:, pw, 2 * tp:2 * tp + 2, :],
                                     gs[r][:, 2 * tp:2 * tp + 2, :],
                                     start=(pw == 0 and tp == 0),
                                     stop=(not with_bias and pw == W2P - 1
                                           and tp == NHT // 2 - 1),
                                     perf_mode=DRm)
            if with_bias:
                nc.tensor.matmul(ps[:], wx2_sb[0:1, co, :, :], xtr_1[0:1, :, :],
                                 start=False, stop=True, perf_mode=DRm)
            ot = opool.tile([128, T], f32, tag="ot", name="ot", bufs=3)
            nc.vector.scalar_tensor_tensor(
                ot[:], ps[:], 1.0 / WS, x2s[r][:, co, :], OP.mult, OP.add)
            nc.sync.dma_start(outT.ap()[r, :, co, :], ot[:])

        # ================= phase 0: loads + LN1 =================
        cm_psln, psln = open_pool("ps_ln", space="PSUM")
        for r in range(RPC):
            nc.sync.dma_start(xs[r][:, 0:CT // 2, :], xT.ap()[r, :, 0:CT // 2, :])
            nc.sync.dma_start(xs[r][:, CT // 2:, :], xT.ap()[r, :, CT // 2:, :])
            nc.sync.dma_start(
                kc_sb[r][:].rearrange("p a b c -> p (a b c)"), kcD.ap()[r])
            nc.sync.dma_start(
                vkv[r][:, 0, :, :, :].rearrange("p h k d -> p (h k d)"),
                vcC.ap()[r])
            nc.gpsimd.memset(vkv[r][:, 1, :, :, HD:HD + 1], 1.0)
            nc.gpsimd.memset(vkv[r][:, 1, :, :, HD + 1:VP], 0.0)
        for r in range(RPC):
            layernorm(xs[r], h1[r], psln, "l1", ones_sb[:, 0:1],
                      lambda ap: ap.bitcast(f32))
        close_pool(cm_psln)

        # ================= qkv r0 + swizzle r0 =================
        cm_wA, wA = open_pool("p_wA")
        cm_psmm, psmm = open_pool("ps_mm", space="PSUM")
        cm_st0, p_st0 = open_pool("p_st0")
        q0 = p_st0.tile([128, CT, T], fp8, tag="q0", name="q0")
        k0 = p_st0.tile([128, CT, T], fp8, tag="k0", name="k0")
        for f in range(4):
            emit_qk_pair(f, 0, wA, psmm, q0, k0)
        emit_swizzle(0, q0, k0, 0)
        for f in range(4, 8):
            emit_qk_pair(f, 0, wA, psmm, q0, k0)
        emit_swizzle(0, q0, k0, 1)
        for ch in range(4):
            emit_v_chunk(ch, 0, wA, psmm)
        close_pool(cm_st0)

        # ================= phase A: attention r0 || qkv r1 =================
        cm_st1, p_st1 = open_pool("p_st1")
        q1 = p_st1.tile([128, CT, T], fp8, tag="q1", name="q1")
        k1 = p_st1.tile([128, CT, T], fp8, tag="k1", name="k1")
        for hp in range(HPAIR):
            emit_attention(hp, 0)
        for f in range(8):
            emit_qk_pair(f, 1, wA, psmm, q1, k1)
            if f % 2 == 1:
                emit_v_chunk(f // 2, 1, wA, psmm)
            if f == 3:
                emit_swizzle(1, q1, k1, 0)
        emit_swizzle(1, q1, k1, 1)
        close_pool(cm_st1)
        close_pool(cm_psmm)
        close_pool(cm_wA)
        close_pool(cm_h1)

        # ====== phase B: attention r1 (priority) || proj/LN2/FC1 r0 ======
        cm_psB, psB = open_pool("ps_B", space="PSUM")
        cm_wB, wB = open_pool("p_wB")
        for hp in range(HPAIR):
            emit_attention(hp, 1)
        for co in range(CT):
            emit_proj(co, 0, wB, psB)
        layernorm(x2s[0], h2[0][:, 0], psB, "l2a", ones_bf[:],
                  lambda ap: ap, dst_lo=h2[0][:, 1])
        for htp in range(NHT // 2):
            emit_fc1_pair(htp, [0], wB, psB)
        close_pool(cm_wB)
        close_pool(cm_psB)
        close_pool(cm_psatt)
        close_pool(cm_att)
        close_pool(cm_kv)

        # ================= tail =================
        with tc.tile_pool(name="ps_T", space="PSUM", bufs=1) as psT, \
             tc.tile_pool(name="p_wT", bufs=1) as wT, \
             tc.tile_pool(name="p_osb", bufs=1) as osb:
            for co in range(CT):
                emit_proj(co, 1, wT, psT)
            layernorm(x2s[1], h2[1][:, 0], psT, "l2b", ones_bf[:],
                      lambda ap: ap, dst_lo=h2[1][:, 1])
            for co in range(CT):
                w2t = wT.tile([128, W2P, NHT, 128], fp8, tag="w2", name="w2", bufs=2)
                nc.sync.dma_start(w2t[:], w2.ap()[co])
                emit_fc2(co, 0, w2t, psT, osb)
            for htp in range(NHT // 2):
                emit_fc1_pair(htp, [1], wT, psT)
            for co in range(CT):
                w2t = wT.tile([128, W2P, NHT, 128], fp8, tag="w2", name="w2", bufs=2)
                nc.sync.dma_start(w2t[:], w2.ap()[co])
                emit_fc2(co, 1, w2t, psT, osb)
        close_pool(cm_h2)
        close_pool(cm_g)
        close_pool(cm_x2)
        close_pool(cm_oT)
        close_pool(cm_xs)
        close_pool(cm_work)

    nc.compile()
    return nc


class _Runner:
    """Hold the compiled PJRT executable (mirrors bass2jax.run_bass_via_pjrt)."""

    def __init__(self, nc, n_cores):
        import jax
        from jax.sharding import Mesh, PartitionSpec
        from jax.experimental.shard_map import shard_map
        import concourse.mybir as mybir
        from concourse.bass2jax import (
            install_neuronx_cc_hook, partition_id_tensor, _bass_exec_p)

        install_neuronx_cc_hook()
        self.jax = jax
        self.n_cores = n_cores
        partition_name = nc.partition_id_tensor.name if nc.partition_id_tensor else None
        in_names, out_names, out_avals, zero_outs = [], [], [], []
        for alloc in nc.m.functions[0].allocations:
            if not isinstance(alloc, mybir.MemoryLocationSet):
                continue
            name = alloc.memorylocations[0].name
            if alloc.kind == "ExternalInput":
                if name != partition_name:
                    in_names.append(name)
            elif alloc.kind == "ExternalOutput":
                shape = tuple(alloc.tensor_shape)
                dtype = mybir.dt.np(alloc.dtype)
                out_names.append(name)
                out_avals.append(jax.core.ShapedArray(shape, dtype))
                zero_outs.append(np.zeros(shape, dtype))
        self.in_names, self.out_names = in_names, out_names
        self.out_avals, self.zero_outs = out_avals, zero_outs
        self.n_params = len(in_names)
        all_names = in_names + out_names
        if partition_name is not None:
            all_names.append(partition_name)

        def _body(*args):
            operands = list(args)
            if partition_name is not None:
                operands.append(partition_id_tensor())
            return tuple(
                _bass_exec_p.bind(
                    *operands,
                    out_avals=tuple(out_avals),
                    in_names=tuple(all_names),
                    out_names=tuple(out_names),
                    lowering_input_output_aliases=(),
                    sim_require_finite=True,
                    sim_require_nnan=True,
                    nc=nc,
                ))

        devices = jax.devices()[:n_cores]
        assert len(devices) == n_cores, f"need {n_cores} cores, have {len(jax.devices())}"
        mesh = Mesh(np.asarray(devices), ("core",))
        n_outs = len(out_names)
        self._fn = jax.jit(
            shard_map(_body, mesh=mesh,
                      in_specs=(PartitionSpec("core"),) * (self.n_params + n_outs),
                      out_specs=(PartitionSpec("core"),) * n_outs,
                      check_rep=False),
            keep_unused=True)

    def prepare(self, in_maps):
        np_ = np
        per_core = [[np_.asarray(m[n]) for n in self.in_names] for m in in_maps]
        concat_in = [
            np_.concatenate([per_core[c][i] for c in range(self.n_cores)], axis=0)
            for i in range(self.n_params)]
        concat_zeros = [
            np_.zeros((self.n_cores * z.shape[0], *z.shape[1:]), z.dtype)
            for z in self.zero_outs]
        return self.jax.device_put(concat_in + concat_zeros)

    def run(self, prepared):
        out = self._fn(*prepared)
        self.jax.block_until_ready(out)
        return out

    def results(self, out_arrs):
        return [
            {name: np.asarray(out_arrs[i]).reshape(
                self.n_cores, *self.out_avals[i].shape)[c]
             for i, name in enumerate(self.out_names)}
            for c in range(self.n_cores)]


def _get_runner(with_bias):
    if "runner" not in _state:
        nc = _build_module(with_bias=with_bias)
        _state["nc"] = nc
        _state["runner"] = _Runner(nc, NCORES)
    return _state["runner"]


def _prepare_in_maps(x, cache_k, cache_v, update_mask, qkv_w, qkv_b, proj_w,
                     proj_b, n1_g, n1_b, n2_g, n2_b, fc1_w, fc1_b, fc2_w, fc2_b):
    f32 = np.float32
    x = np.asarray(x, f32)
    cache_k = np.asarray(cache_k, f32)
    cache_v = np.asarray(cache_v, f32)
    update_mask = np.asarray(update_mask, bool)
    qkv_w = np.asarray(qkv_w, f32)
    qkv_b = np.asarray(qkv_b, f32)
    proj_w = np.asarray(proj_w, f32)
    proj_b = np.asarray(proj_b, f32)
    n1_g = np.asarray(n1_g, f32)
    n1_b = np.asarray(n1_b, f32)
    n2_g = np.asarray(n2_g, f32)
    n2_b = np.asarray(n2_b, f32)
    fc1_w = np.asarray(fc1_w, f32)
    fc1_b = np.asarray(fc1_b, f32)
    fc2_w = np.asarray(fc2_w, f32)
    fc2_b = np.asarray(fc2_b, f32)

    xT = np.ascontiguousarray(
        x.transpose(0, 2, 1).reshape(B, CT, 128, T).transpose(0, 2, 1, 3))

    kcD = np.empty((B, 128, 2, 4, NKC), fp8np)
    vcC = np.empty((B, 128, H, KTC, VP), fp8np)
    for b in range(B):
        keep = ~update_mask[b]
        kc = cache_k[b][:, keep, :]          # [H, NKC, HD]
        vc = cache_v[b][:, keep, :]
        kt_ = kc.transpose(0, 2, 1)          # [H, HD, NKC]
        for h in range(H):
            d_ = 2 * (h % 2) + (h // 2) // 4
            hg_ = (h // 2) % 4
            for half in range(2):
                kcD[b, 32 * d_:32 * d_ + 32, half, hg_, :] = (
                    kt_[h, 32 * half:32 * half + 32, :].astype(fp8np))
        vv = (vc.transpose(1, 0, 2).reshape(KTC, 128, H, HD)
              .transpose(1, 2, 0, 3))        # [128, H, KTC, HD]
        pad = np.zeros((128, H, KTC, VP - HD), f32)
        pad[:, :, :, 0] = 1.0
        vcC[b] = np.concatenate([vv * OS, pad], axis=3).astype(fp8np)

    def wtile(w, nf, cols):
        ci = w.shape[0]
        return np.ascontiguousarray(
            (WS * w).reshape(ci // 128, 128, nf, cols)
            .transpose(2, 1, 0, 3)).astype(fp8np)

    def wtile_hl(w, nf, cols):
        ws = WS * w
        hi = ws.astype(fp8np)
        lo = (ws - hi.astype(np.float32)).astype(fp8np)
        ci = w.shape[0]

        def t(a):
            return (a.reshape(ci // 128, 128, nf, cols).transpose(2, 1, 0, 3))
        return np.ascontiguousarray(
            np.stack([t(hi), t(lo)], axis=2)).astype(fp8np)

    wqkv_eff = n1_g[:, None] * qkv_w
    bias_qkv = n1_b @ qkv_w + qkv_b
    wqk16 = wtile(wqkv_eff[:, :2048], 16, 128)       # [16,128,CT,128]
    wqk_t = np.empty((8, 128, 2, CT, 128), fp8np)
    for f in range(8):
        wqk_t[f, :, 0] = wqk16[f]
        wqk_t[f, :, 1] = wqk16[8 + f]
    wv_t = wtile(wqkv_eff[:, 2048:], 4, 256)
    bqk_t = np.ascontiguousarray(bias_qkv[:2048].reshape(16, 128).T).astype(f32)
    vbias_t = (OS * bias_qkv[2048:]).astype(f32)

    wp_t = wtile(proj_w, CT, 128)
    wxp_t = np.zeros((1, CT, 2, 128), fp8np)
    wxp_t[0, :, 0, :] = (WS * proj_b).reshape(CT, 128).astype(fp8np)

    w1_eff = n2_g[:, None] * fc1_w
    bias_fc1 = n2_b @ fc1_w + fc1_b
    w1_hl = wtile_hl(w1_eff, NHT, 128)               # [32,128,2,CT,128]
    w1_t = np.ascontiguousarray(
        w1_hl.reshape(NHT // 2, 2, 128, 2, CT, 128)
        .transpose(0, 2, 1, 3, 4, 5))                # [16,128,2,2,CT,128]
    wx1_t = np.zeros((1, NHT, 2, 128), fp8np)
    wx1_t[0, :, 0, :] = (WS * bias_fc1).reshape(NHT, 128).astype(fp8np)

    w2_t = np.ascontiguousarray(
        wtile_hl(fc2_w, CT, 128)[:, :, :W2P])        # [CT,128,W2P,NHT,128]
    wx2_t = np.zeros((1, CT, 2, 128), fp8np)
    wx2_t[0, :, 0, :] = (WS * fc2_b).reshape(CT, 128).astype(fp8np)

    with_bias = bool(np.any(wxp_t) or np.any(wx1_t) or np.any(wx2_t))
    shared = dict(
        wqk=wqk_t, wv=wv_t, wp=wp_t, w1=w1_t, w2=w2_t,
        bqk=bqk_t, vbias=vbias_t,
        ones=np.ones((128, 1), f32),
    )
    if with_bias:
        shared.update(wxp=wxp_t, wx1=wx1_t, wx2=wx2_t)
    in_maps = []
    for c in range(NCORES):
        s = slice(c * RPC, (c + 1) * RPC)
        in_maps.append(dict(
            shared, xT=xT[s], kcD=kcD[s],
            vcC=vcC[s].reshape(RPC, 128, H * KTC * VP)))
    return in_maps, with_bias


def kernel(**inputs) -> np.ndarray:
    in_maps, with_bias = _prepare_in_maps(**inputs)
    runner = _get_runner(with_bias)
    prepared = runner.prepare(in_maps)
    out = runner.run(prepared)
    res = runner.results(out)
    full = np.empty((B, NP, C), np.float32)
    for c in range(NCORES):
        for r in range(RPC):
            full[c * RPC + r] = res[c]["outT"][r].transpose(2, 1, 0).reshape(T, C)
    return full


# revision 34
# speedup vs baseline: 1.8041x; 1.0160x over previous
"""Fused decoder block (LN->QKV->cache-merge attention->proj->LN->MLP) on 8
Trainium2 NeuronCores, data-parallel over the batch (2 rows/core).

v3: row-pipelined schedule. The two batch rows per core are independent, so
the ACT-bound attention of row 1 overlaps the PE-bound proj/LN2/FC1 of row 0.

Key ideas (cumulative):
- host-side cache compaction via update_mask: attention over [512 surviving
  cache keys ++ 512 new keys] = 1024 keys, no mask bias (softmax is
  permutation invariant).
- fp8e4 DoubleRow matmuls everywhere (QKV, scores, PV, proj, FC1, FC2);
  weights host-quantized at x64 scale; hi+lo residual passes for W1/W2/h2
  keep the MLP path accurate (rel err ~1.3e-2 < 2e-2).
- q/k swizzled into [32, 2, head, T] pair layout by SBUF->SBUF DMAs so the
  64-dim head contraction runs as DoubleRow (half cost).
- LN standardization only (gains/biases folded into weights host-side);
  sum-sq stats via fp8 DR on squared tiles; rstd/-mu broadcast across
  partitions by GPSIMD partition_broadcast (no PE, no PSUM).
- linear biases as extra fp8 contraction rows; softmax denominator from a
  ones column in V, reciprocal broadcast on Pool.
- x2 residual kept in bf16 to fit SBUF; PSUM pools sized to exactly 8 banks
  per phase.
"""

import numpy as np
import ml_dtypes

B, NP, N, C, H = 16, 512, 1024, 1024, 16
HD = C // H            # 64
HID = 4 * C            # 4096
EPS = 1e-5
NCORES = 8
RPC = B // NCORES      # batch rows per core
T = NP                 # queries per row
CT = C // 128          # feature tiles
NKC = N - NP           # surviving cache keys (512)
KTC = NKC // 128       # cache key tiles (4)
KTN = T // 128         # new key tiles (4)
KT = KTC + KTN         # total key tiles (8)
HPAIR = H // 2
NHT = HID // 128       # fc1 output chunks (32)
SCALE = HD ** -0.5
WS = 64.0              # weight quantization scale
OS = 16.0              # v / attention-output scale
VP = 80                # padded v row (64 d + 1 ones + 15 pad)
W2P = 1                # fc2 weight passes (1 = hi only, 2 = hi+lo residual)

_state = {}
fp8np = ml_dtypes.float8_e4m3


def _build_module(with_bias=True):
    import concourse.tile as tile
    from concourse import bacc, mybir

    f32 = mybir.dt.float32
    f32r = mybir.dt.float32r
    bf16 = mybir.dt.bfloat16
    fp8 = mybir.dt.float8e4
    AF = mybir.ActivationFunctionType
    OP = mybir.AluOpType
    DRm = mybir.MatmulPerfMode.DoubleRow

    nc = bacc.Bacc("TRN2", target_bir_lowering=False, debug=False)

    xT = nc.dram_tensor("xT", [RPC, 128, CT, T], f32r, kind="ExternalInput")
    kcD = nc.dram_tensor("kcD", [RPC, 128, 2, 4, NKC], fp8, kind="ExternalInput")
    vcC = nc.dram_tensor("vcC", [RPC, 128, H * KTC * VP], fp8, kind="ExternalInput")
    wqk = nc.dram_tensor("wqk", [8, 128, 2, CT, 128], fp8, kind="ExternalInput")
    wv = nc.dram_tensor("wv", [4, 128, CT, 256], fp8, kind="ExternalInput")
    wp = nc.dram_tensor("wp", [CT, 128, CT, 128], fp8, kind="ExternalInput")
    w1 = nc.dram_tensor("w1", [NHT // 2, 128, 2, 2, CT, 128], fp8,
                        kind="ExternalInput")
    w2 = nc.dram_tensor("w2", [CT, 128, W2P, NHT, 128], fp8, kind="ExternalInput")
    if with_bias:
        wxp = nc.dram_tensor("wxp", [1, CT, 2, 128], fp8, kind="ExternalInput")
        wx1 = nc.dram_tensor("wx1", [1, NHT, 2, 128], fp8, kind="ExternalInput")
        wx2 = nc.dram_tensor("wx2", [1, CT, 2, 128], fp8, kind="ExternalInput")
    bqk = nc.dram_tensor("bqk", [128, 16], f32, kind="ExternalInput")
    vbias = nc.dram_tensor("vbias", [C], f32, kind="ExternalInput")
    ones = nc.dram_tensor("ones", [128, 1], f32r, kind="ExternalInput")
    outT = nc.dram_tensor("outT", [RPC, 128, CT, T], f32, kind="ExternalOutput")

    from contextlib import ExitStack
    with nc.allow_low_precision(reason="deliberate fp8/f32r staging; PSUM accumulation fp32"), \
         tile.TileContext(nc, pool_alloc_mode="queue") as tc, ExitStack() as es:
        # ---------- constants ----------
        consts = es.enter_context(tc.tile_pool(name="consts", bufs=1))
        ones_sb = consts.tile([128, 1], f32r)
        nc.sync.dma_start(ones_sb[:], ones.ap())
        ones_bf = consts.tile([128, 1], bf16)
        nc.vector.memset(ones_bf[:], 1.0)
        ones8 = consts.tile([128, 2, 16], fp8)
        nc.vector.memset(ones8[:], 1.0)
        bqk_sb = consts.tile([128, 16], f32)
        nc.sync.dma_start(bqk_sb[:], bqk.ap())
        if with_bias:
            vb_sb = consts.tile([128, C], bf16)
            nc.gpsimd.dma_start(vb_sb[:], vbias.ap()[None].to_broadcast((128, C)))
        if with_bias:
            wxp_sb = consts.tile([1, CT, 2, 128], fp8)
            nc.sync.dma_start(wxp_sb[:], wxp.ap())
            wx1_sb = consts.tile([1, NHT, 2, 128], fp8)
            nc.sync.dma_start(wx1_sb[:], wx1.ap())
            wx2_sb = consts.tile([1, CT, 2, 128], fp8)
            nc.sync.dma_start(wx2_sb[:], wx2.ap())
            xtr_p = consts.tile([1, 2, 512], fp8)
            nc.vector.memset(xtr_p[0:1, 0, :], OS)
            nc.vector.memset(xtr_p[0:1, 1, :], 0.0)
            xtr_1 = consts.tile([1, 2, 512], fp8)
            nc.vector.memset(xtr_1[0:1, 0, :], 1.0)
            nc.vector.memset(xtr_1[0:1, 1, :], 0.0)
        eps_sb = consts.tile([1, 1], f32)
        nc.vector.memset(eps_sb[:], EPS)

        def open_pool(nm, space=None):
            kw = dict(space=space) if space else {}
            cm = tc.tile_pool(name=nm, bufs=1, **kw)
            return cm, cm.__enter__()

        def close_pool(cm):
            cm.__exit__(None, None, None)

        # ---------- long-lived pools (ring-stack order) ----------
        cm_work, p_work = open_pool("p_work")
        cm_xs, p_xs = open_pool("p_xs")
        xs = [p_xs.tile([128, CT, T], f32r, tag=f"xs{r}", name=f"xs{r}")
              for r in range(RPC)]
        cm_oT, p_oT = open_pool("p_oT")
        oT = [p_oT.tile([128, CT, T], fp8, tag=f"oT{r}", name=f"oT{r}")
              for r in range(RPC)]
        cm_x2, p_x2 = open_pool("p_x2")
        x2s = [p_x2.tile([128, CT, T], bf16, tag=f"x2{r}", name=f"x2{r}")
               for r in range(RPC)]
        cm_g, p_g = open_pool("p_g")
        gs = [p_g.tile([128, NHT, T], fp8, tag=f"g{r}", name=f"g{r}")
              for r in range(RPC)]
        cm_h2, p_h2 = open_pool("p_h2")
        h2 = [p_h2.tile([128, 2, CT, T], fp8, tag=f"h2{r}", name=f"h2{r}")
              for r in range(RPC)]
        cm_kv, p_kv = open_pool("p_kv")
        kc_sb = [p_kv.tile([128, 2, 4, NKC], fp8, tag=f"kc{r}", name=f"kc{r}")
                 for r in range(RPC)]
        vkv = [p_kv.tile([128, 2, H, KTC, VP], fp8, tag=f"vkv{r}", name=f"vkv{r}")
               for r in range(RPC)]
        qD = [p_kv.tile([128, 2, 4, T], fp8, tag=f"qD{r}", name=f"qD{r}")
              for r in range(RPC)]
        kD = [p_kv.tile([128, 2, 4, T], fp8, tag=f"kD{r}", name=f"kD{r}")
              for r in range(RPC)]
        cm_att, apool = open_pool("p_att")
        cm_psatt, psatt = open_pool("ps_att", space="PSUM")
        cm_h1, p_h1 = open_pool("p_h1")
        h1 = [p_h1.tile([128, CT, T], fp8, tag=f"h1{r}", name=f"h1{r}")
              for r in range(RPC)]

        # ================= layernorm =================
        def layernorm(src, dst, stats_pool, st_tag, lhs_ones, src_cast,
                      dst_lo=None, stats_tag="misc"):
            s_t = stats_pool.tile([128, T], f32, tag=stats_tag,
                                  name=f"{st_tag}s", bufs=2)
            s_ps = s_t[0:1, :]
            for ct in range(CT):
                nc.tensor.matmul(s_ps, lhs_ones, src[:, ct, :],
                                 start=(ct == 0), stop=(ct == CT - 1))
            ss_t = stats_pool.tile([128, T], f32, tag=stats_tag,
                                   name=f"{st_tag}ss", bufs=2)
            ss_ps = ss_t[0:1, :]
            for c in range(CT // 2):
                sqs = p_work.tile([128, 2, T], fp8, tag="sqs", name="sqs", bufs=1)
                for j in range(2):
                    eng = nc.gpsimd if j else nc.vector
                    eng.tensor_mul(sqs[:, j, :], src_cast(src[:, 2 * c + j, :]),
                                   src_cast(src[:, 2 * c + j, :]))
                nc.tensor.matmul(ss_ps, ones8[:, :, 0:1], sqs[:],
                                 start=(c == 0), stop=(c == CT // 2 - 1),
                                 perf_mode=DRm)
            st = p_work.tile([97, T], f32, tag="st", name="st", bufs=1)
            negmu, msq, var, stdv = (st[0:1, :], st[32:33, :], st[64:65, :],
                                     st[96:97, :])
            nc.vector.tensor_scalar(negmu, s_ps, -1.0 / C, None, OP.mult)
            nc.vector.tensor_mul(msq, negmu, negmu)
            nc.vector.scalar_tensor_tensor(var, ss_ps, 1.0 / C, msq,
                                           OP.mult, OP.subtract)
            nc.scalar.activation(stdv, var, AF.Sqrt, bias=eps_sb[:])
            rstd = p_work.tile([1, T], f32, tag="rstd", name="rstd", bufs=2)
            nc.vector.reciprocal(rstd[:], stdv)
            A_sb = p_work.tile([128, T], f32, tag="Asb", name="Asb", bufs=1)
            nc.gpsimd.partition_broadcast(A_sb[:], rstd[:])
            M_sb = p_work.tile([128, T], f32, tag="Msb", name="Msb", bufs=1)
            nc.gpsimd.partition_broadcast(M_sb[:], negmu)
            for ct in range(CT):
                tmp = p_work.tile([128, T], f32, tag="tmp", name="tmp", bufs=2)
                e1, e2 = ((nc.gpsimd, nc.vector) if ct % 3 == 2
                          else (nc.vector, nc.gpsimd)
                          if ct % 3 == 1 else (nc.vector, nc.vector))
                e1.tensor_add(tmp[:], src_cast(src[:, ct, :]), M_sb[:])
                if dst_lo is None:
                    e2.tensor_mul(dst[:, ct, :], tmp[:], A_sb[:])
                else:
                    e2.tensor_mul(tmp[:], tmp[:], A_sb[:])
                    e1.tensor_copy(dst[:, ct, :], tmp[:])
                    nc.vector.scalar_tensor_tensor(
                        dst_lo[:, ct, :], dst[:, ct, :], -1.0, tmp[:],
                        OP.mult, OP.add)

        # ================= emitters =================
        def emit_qk_pair(f, r, wpool, mmps, qsb, ksb):
            wt = wpool.tile([128, 2, CT, 128], fp8, tag="wqk", name="wqk", bufs=3)
            nc.sync.dma_start(wt[:], wqk.ap()[f])
            for which in range(2):
                ps = mmps.tile([128, T], f32, tag="mm", name="mm", bufs=2)
                for c in range(CT // 2):
                    nc.tensor.matmul(ps[:], wt[:, which, 2 * c:2 * c + 2, :],
                                     h1[r][:, 2 * c:2 * c + 2, :],
                                     start=(c == 0), stop=(c == CT // 2 - 1),
                                     perf_mode=DRm)
                dst = (qsb if which == 0 else ksb)[:, f, :]
                nc.vector.tensor_scalar(dst, ps[:], 1.0 / WS,
                                        bqk_sb[:, 8 * which + f:8 * which + f + 1],
                                        OP.mult, OP.add)

        def emit_v_chunk(ch, r, wpool, mmps):
            wvt = wpool.tile([128, CT, 256], fp8, tag="wv", name="wv", bufs=2)
            nc.sync.dma_start(wvt[:], wv.ap()[ch])
            for tt in range(KTN):
                psf = mmps.tile([128, T], f32, tag="mm", name="mm", bufs=2)
                ps = psf[:, 0:256]
                for c in range(CT // 2):
                    nc.tensor.matmul(
                        ps, h1[r][:, 2 * c:2 * c + 2, tt * 128:(tt + 1) * 128],
                        wvt[:, 2 * c:2 * c + 2, :],
                        start=(c == 0), stop=(c == CT // 2 - 1), perf_mode=DRm)
                if with_bias:
                    nc.vector.scalar_tensor_tensor(
                        vkv[r][:, 1, 4 * ch:4 * ch + 4, tt, 0:HD],
                        ps.rearrange("p (h d) -> p h d", h=4), OS / WS,
                        vb_sb[:, ch * 256:(ch + 1) * 256]
                        .rearrange("p (h d) -> p h d", h=4),
                        OP.mult, OP.add)
                else:
                    nc.vector.tensor_scalar(
                        vkv[r][:, 1, 4 * ch:4 * ch + 4, tt, 0:HD],
                        ps.rearrange("p (h d) -> p h d", h=4), OS / WS,
                        None, OP.mult)

        def emit_swizzle(r, qsb, ksb, fq):
            # head h=2f+par -> block d=2*par+f//4, hg=f%4
            for src_sb, dst in ((qsb, qD[r]), (ksb, kD[r])):
                for par in range(2):
                    for half in range(2):
                        nc.sync.dma_start(
                            dst[32 * (2 * par + fq):32 * (2 * par + fq) + 32,
                                half, :, :],
                            src_sb[64 * par + 32 * half:
                                   64 * par + 32 * half + 32,
                                   4 * fq:4 * fq + 4, :])

        def emit_attention(hp, r):
            for hh in range(2):
                h = 2 * hp + hh
                d = 2 * (h % 2) + (h // 2) // 4
                hg = (h // 2) % 4
                pb = 32 * d
                pt = apool.tile([128, KT, T], fp8, tag="pt", name="pt", bufs=2)
                pv = psatt.tile([128, T], f32, tag="pv", name="pv", bufs=2)
                for g in range(KT // 2):
                    sc = psatt.tile([128, 2, T], f32, tag="sc", name="sc", bufs=2)
                    for j in range(2):
                        kt = 2 * g + j
                        if kt < KTC:
                            lhs = kc_sb[r][pb:pb + 32, :, hg,
                                           kt * 128:(kt + 1) * 128]
                        else:
                            lhs = kD[r][pb:pb + 32, :, hg,
                                        (kt - KTC) * 128:(kt - KTC + 1) * 128]
                        nc.tensor.matmul(sc[:, j, :], lhs,
                                         qD[r][pb:pb + 32, :, hg, :],
                                         start=True, stop=True, perf_mode=DRm,
                                         tile_position=(pb, 0))
                    nc.scalar.activation(
                        pt[:, 2 * g:2 * g + 2, :].rearrange("p a b -> p (a b)"),
                        sc[:].rearrange("p a b -> p (a b)"), AF.Exp, scale=SCALE)
                    half = 0 if g < KTC // 2 else 1
                    koff = 0 if g < KTC // 2 else KTC
                    nc.tensor.matmul(
                        pv[0:VP, :],
                        vkv[r][:, half, h, 2 * g - koff:2 * g - koff + 2, :],
                        pt[:, 2 * g:2 * g + 2, :],
                        start=(g == 0), stop=(g == KT // 2 - 1), perf_mode=DRm)
                rd = apool.tile([1, T], f32, tag="rd", name="rd", bufs=2)
                nc.vector.reciprocal(rd[:], pv[HD:HD + 1, :])
                bcs = apool.tile([HD, T], f32, tag="bcs", name="bcs", bufs=2)
                nc.gpsimd.partition_broadcast(bcs[:], rd[:])
                nc.vector.tensor_mul(oT[r][64 * hh:64 * hh + 64, hp, :],
                                     pv[0:HD, :], bcs[:])

        def emit_proj(co, r, wpool, mpool):
            wt = wpool.tile([128, CT, 128], fp8, tag="wp", name="wp", bufs=3)
            nc.sync.dma_start(wt[:], wp.ap()[co])
            ps = mpool.tile([128, T], f32, tag="misc", name="misc", bufs=2)
            for c in range(CT // 2):
                nc.tensor.matmul(ps[:], wt[:, 2 * c:2 * c + 2, :],
                                 oT[r][:, 2 * c:2 * c + 2, :],
                                 start=(c == 0),
                                 stop=(not with_bias and c == CT // 2 - 1),
                                 perf_mode=DRm)
            if with_bias:
                nc.tensor.matmul(ps[:], wxp_sb[0:1, co, :, :], xtr_p[0:1, :, :],
                                 start=False, stop=True, perf_mode=DRm)
            nc.vector.scalar_tensor_tensor(
                x2s[r][:, co, :], ps[:], 1.0 / (OS * WS),
                xs[r][:, co, :].bitcast(f32), OP.mult, OP.add)

        def emit_fc1_pair(htp, rows, wpool, mpool):
            wt = wpool.tile([128, 2, 2, CT, 128], fp8, tag="w1", name="w1", bufs=3)
            nc.sync.dma_start(wt[:], w1.ap()[htp])
            for r in rows:
                for j in range(2):
                    ht = 2 * htp + j
                    ps = mpool.tile([128, T], f32, tag="misc", name="misc", bufs=2)
                    for pi, (pw, ph) in enumerate(((0, 0), (0, 1), (1, 0))):
                        for c in range(CT // 2):
                            nc.tensor.matmul(ps[:],
                                             wt[:, j, pw, 2 * c:2 * c + 2, :],
                                             h2[r][:, ph, 2 * c:2 * c + 2, :],
                                             start=(pi == 0 and c == 0),
                                             stop=(not with_bias and pi == 2
                                                   and c == CT // 2 - 1),
                                             perf_mode=DRm)
                    if with_bias:
                        nc.tensor.matmul(ps[:], wx1_sb[0:1, ht, :, :],
                                         xtr_1[0:1, :, :],
                                         start=False, stop=True, perf_mode=DRm)
                    nc.scalar.activation(gs[r][:, ht, :], ps[:], AF.Gelu,
                                         scale=1.0 / WS)

        def emit_fc2(co, r, wt, f2pool, opool):
            ps = f2pool.tile([128, T], f32, tag="fc2", name="fc2", bufs=3)
            for pw in range(W2P):
                for tp in range(NHT // 2):
                    nc.tensor.matmul(ps[:], wt[:, pw, 2 * tp:2 * tp + 2, :],
                                     gs[r][:, 2 * tp:2 * tp + 2, :],
                                     start=(pw == 0 and tp == 0),
                                     stop=(not with_bias and pw == W2P - 1
                                           and tp == NHT // 2 - 1),
                                     perf_mode=DRm)
            if with_bias:
                nc.tensor.matmul(ps[:], wx2_sb[0:1, co, :, :], xtr_1[0:1, :, :],
                                 start=False, stop=True, perf_mode=DRm)
            ot = opool.tile([128, T], f32, tag="ot", name="ot", bufs=3)
            nc.vector.scalar_tensor_tensor(
                ot[:], ps[:], 1.0 / WS, x2s[r][:, co, :], OP.mult, OP.add)
            nc.sync.dma_start(outT.ap()[r, :, co, :], ot[:])

        # ================= phase 0: loads + LN1 =================
        cm_psln, psln = open_pool("ps_ln", space="PSUM")
        for r in range(RPC):
            nc.sync.dma_start(xs[r][:, 0:CT // 2, :], xT.ap()[r, :, 0:CT // 2, :])
            nc.sync.dma_start(xs[r][:, CT // 2:, :], xT.ap()[r, :, CT // 2:, :])
            nc.sync.dma_start(
                kc_sb[r][:].rearrange("p a b c -> p (a b c)"), kcD.ap()[r])
            nc.sync.dma_start(
                vkv[r][:, 0, :, :, :].rearrange("p h k d -> p (h k d)"),
                vcC.ap()[r])
            nc.gpsimd.memset(vkv[r][:, 1, :, :, HD:HD + 1], 1.0)
            nc.gpsimd.memset(vkv[r][:, 1, :, :, HD + 1:VP], 0.0)
        layernorm(xs[0], h1[0], psln, "l1", ones_sb[:, 0:1],
                  lambda ap: ap.bitcast(f32))
        close_pool(cm_psln)

        # ================= qkv r0 + swizzle r0 (warmup-critical) =========
        cm_wA, wA = open_pool("p_wA")
        cm_psmm, psmm = open_pool("ps_mm", space="PSUM")
        cm_st0, p_st0 = open_pool("p_st0")
        q0 = p_st0.tile([128, CT, T], fp8, tag="q0", name="q0")
        k0 = p_st0.tile([128, CT, T], fp8, tag="k0", name="k0")
        for f in range(4):
            emit_qk_pair(f, 0, wA, psmm, q0, k0)
        emit_swizzle(0, q0, k0, 0)
        for f in range(4, 8):
            emit_qk_pair(f, 0, wA, psmm, q0, k0)
        emit_swizzle(0, q0, k0, 1)
        emit_v_chunk(0, 0, wA, psmm)
        emit_v_chunk(1, 0, wA, psmm)
        # r1's LN1 is not needed until phase A -- emit after r0's warmup chain
        layernorm(xs[1], h1[1], psmm, "l1b", ones_sb[:, 0:1],
                  lambda ap: ap.bitcast(f32), stats_tag="mm")
        emit_v_chunk(2, 0, wA, psmm)
        emit_v_chunk(3, 0, wA, psmm)
        close_pool(cm_st0)

        # ================= phase A: attention r0 || qkv r1 =================
        cm_st1, p_st1 = open_pool("p_st1")
        q1 = p_st1.tile([128, CT, T], fp8, tag="q1", name="q1")
        k1 = p_st1.tile([128, CT, T], fp8, tag="k1", name="k1")
        for hp in range(HPAIR):
            emit_attention(hp, 0)
        for f in range(8):
            emit_qk_pair(f, 1, wA, psmm, q1, k1)
            if f % 2 == 1:
                emit_v_chunk(f // 2, 1, wA, psmm)
            if f == 3:
                emit_swizzle(1, q1, k1, 0)
        emit_swizzle(1, q1, k1, 1)
        close_pool(cm_st1)
        close_pool(cm_psmm)
        close_pool(cm_wA)
        close_pool(cm_h1)

        # ====== phase B: attention r1 (priority) || proj/LN2/FC1 r0 ======
        cm_psB, psB = open_pool("ps_B", space="PSUM")
        cm_wB, wB = open_pool("p_wB")
        for hp in range(HPAIR):
            emit_attention(hp, 1)
        for co in range(CT):
            emit_proj(co, 0, wB, psB)
        layernorm(x2s[0], h2[0][:, 0], psB, "l2a", ones_bf[:],
                  lambda ap: ap, dst_lo=h2[0][:, 1])
        for htp in range(NHT // 2):
            emit_fc1_pair(htp, [0], wB, psB)
        close_pool(cm_wB)
        close_pool(cm_psB)
        close_pool(cm_psatt)
        close_pool(cm_att)
        close_pool(cm_kv)

        # ================= tail =================
        with tc.tile_pool(name="ps_T", space="PSUM", bufs=1) as psT, \
             tc.tile_pool(name="p_wT", bufs=1) as wT, \
             tc.tile_pool(name="p_osb", bufs=1) as osb:
            for co in range(CT):
                emit_proj(co, 1, wT, psT)
            layernorm(x2s[1], h2[1][:, 0], psT, "l2b", ones_bf[:],
                      lambda ap: ap, dst_lo=h2[1][:, 1])
            for co in range(CT):
                w2t = wT.tile([128, W2P, NHT, 128], fp8, tag="w2", name="w2", bufs=2)
                nc.sync.dma_start(w2t[:], w2.ap()[co])
                emit_fc2(co, 0, w2t, psT, osb)
            for htp in range(NHT // 2):
                emit_fc1_pair(htp, [1], wT, psT)
            for co in range(CT):
                w2t = wT.tile([128, W2P, NHT, 128], fp8, tag="w2", name="w2", bufs=2)
                nc.sync.dma_start(w2t[:], w2.ap()[co])
                emit_fc2(co, 1, w2t, psT, osb)
        close_pool(cm_h2)
        close_pool(cm_g)
        close_pool(cm_x2)
        close_pool(cm_oT)
        close_pool(cm_xs)
        close_pool(cm_work)

    nc.compile()
    return nc


class _Runner:
    """Hold the compiled PJRT executable (mirrors bass2jax.run_bass_via_pjrt)."""

    def __init__(self, nc, n_cores):
        import jax
        from jax.sharding import Mesh, PartitionSpec
        from jax.experimental.shard_map import shard_map
        import concourse.mybir as mybir
        from concourse.bass2jax import (
            install_neuronx_cc_hook, partition_id_tensor, _bass_exec_p)

        install_neuronx_cc_hook()
        self.jax = jax
        self.n_cores = n_cores
        partition_name = nc.partition_id_tensor.name if nc.partition_id_tensor else None
        in_names, out_names, out_avals, zero_outs = [], [], [], []
        for alloc in nc.m.functions[0].allocations:
            if not isinstance(alloc, mybir.MemoryLocationSet):
                continue
            name = alloc.memorylocations[0].name
            if alloc.kind == "ExternalInput":
                if name != partition_name:
                    in_names.append(name)
            elif alloc.kind == "ExternalOutput":
                shape = tuple(alloc.tensor_shape)
                dtype = mybir.dt.np(alloc.dtype)
                out_names.append(name)
                out_avals.append(jax.core.ShapedArray(shape, dtype))
                zero_outs.append(np.zeros(shape, dtype))
        self.in_names, self.out_names = in_names, out_names
        self.out_avals, self.zero_outs = out_avals, zero_outs
        self.n_params = len(in_names)
        all_names = in_names + out_names
        if partition_name is not None:
            all_names.append(partition_name)

        def _body(*args):
            operands = list(args)
            if partition_name is not None:
                operands.append(partition_id_tensor())
            return tuple(
                _bass_exec_p.bind(
                    *operands,
                    out_avals=tuple(out_avals),
                    in_names=tuple(all_names),
                    out_names=tuple(out_names),
                    lowering_input_output_aliases=(),
                    sim_require_finite=True,
                    sim_require_nnan=True,
                    nc=nc,
                ))

        devices = jax.devices()[:n_cores]
        assert len(devices) == n_cores, f"need {n_cores} cores, have {len(jax.devices())}"
        mesh = Mesh(np.asarray(devices), ("core",))
        n_outs = len(out_names)
        self._fn = jax.jit(
            shard_map(_body, mesh=mesh,
                      in_specs=(PartitionSpec("core"),) * (self.n_params + n_outs),
                      out_specs=(PartitionSpec("core"),) * n_outs,
                      check_rep=False),
            keep_unused=True)

    def prepare(self, in_maps):
        np_ = np
        per_core = [[np_.asarray(m[n]) for n in self.in_names] for m in in_maps]
        concat_in = [
            np_.concatenate([per_core[c][i] for c in range(self.n_cores)], axis=0)
            for i in range(self.n_params)]
        concat_zeros = [
            np_.zeros((self.n_cores * z.shape[0], *z.shape[1:]), z.dtype)
            for z in self.zero_outs]
        return self.jax.device_put(concat_in + concat_zeros)

    def run(self, prepared):
        out = self._fn(*prepared)
        self.jax.block_until_ready(out)
        return out

    def results(self, out_arrs):
        return [
            {name: np.asarray(out_arrs[i]).reshape(
                self.n_cores, *self.out_avals[i].shape)[c]
             for i, name in enumerate(self.out_names)}
            for c in range(self.n_cores)]


def _get_runner(with_bias):
    if "runner" not in _state:
        nc = _build_module(with_bias=with_bias)
        _state["nc"] = nc
        _state["runner"] = _Runner(nc, NCORES)
    return _state["runner"]


def _prepare_in_maps(x, cache_k, cache_v, update_mask, qkv_w, qkv_b, proj_w,
                     proj_b, n1_g, n1_b, n2_g, n2_b, fc1_w, fc1_b, fc2_w, fc2_b):
    f32 = np.float32
    x = np.asarray(x, f32)
    cache_k = np.asarray(cache_k, f32)
    cache_v = np.asarray(cache_v, f32)
    update_mask = np.asarray(update_mask, bool)
    qkv_w = np.asarray(qkv_w, f32)
    qkv_b = np.asarray(qkv_b, f32)
    proj_w = np.asarray(proj_w, f32)
    proj_b = np.asarray(proj_b, f32)
    n1_g = np.asarray(n1_g, f32)
    n1_b = np.asarray(n1_b, f32)
    n2_g = np.asarray(n2_g, f32)
    n2_b = np.asarray(n2_b, f32)
    fc1_w = np.asarray(fc1_w, f32)
    fc1_b = np.asarray(fc1_b, f32)
    fc2_w = np.asarray(fc2_w, f32)
    fc2_b = np.asarray(fc2_b, f32)

    xT = np.ascontiguousarray(
        x.transpose(0, 2, 1).reshape(B, CT, 128, T).transpose(0, 2, 1, 3))

    kcD = np.empty((B, 128, 2, 4, NKC), fp8np)
    vcC = np.empty((B, 128, H, KTC, VP), fp8np)
    for b in range(B):
        keep = ~update_mask[b]
        kc = cache_k[b][:, keep, :]          # [H, NKC, HD]
        vc = cache_v[b][:, keep, :]
        kt_ = kc.transpose(0, 2, 1)          # [H, HD, NKC]
        for h in range(H):
            d_ = 2 * (h % 2) + (h // 2) // 4
            hg_ = (h // 2) % 4
            for half in range(2):
                kcD[b, 32 * d_:32 * d_ + 32, half, hg_, :] = (
                    kt_[h, 32 * half:32 * half + 32, :].astype(fp8np))
        vv = (vc.transpose(1, 0, 2).reshape(KTC, 128, H, HD)
              .transpose(1, 2, 0, 3))        # [128, H, KTC, HD]
        pad = np.zeros((128, H, KTC, VP - HD), f32)
        pad[:, :, :, 0] = 1.0
        vcC[b] = np.concatenate([vv * OS, pad], axis=3).astype(fp8np)

    def wtile(w, nf, cols):
        ci = w.shape[0]
        return np.ascontiguousarray(
            (WS * w).reshape(ci // 128, 128, nf, cols)
            .transpose(2, 1, 0, 3)).astype(fp8np)

    def wtile_hl(w, nf, cols):
        ws = WS * w
        hi = ws.astype(fp8np)
        lo = (ws - hi.astype(np.float32)).astype(fp8np)
        ci = w.shape[0]

        def t(a):
            return (a.reshape(ci // 128, 128, nf, cols).transpose(2, 1, 0, 3))
        return np.ascontiguousarray(
            np.stack([t(hi), t(lo)], axis=2)).astype(fp8np)

    wqkv_eff = n1_g[:, None] * qkv_w
    bias_qkv = n1_b @ qkv_w + qkv_b
    wqk16 = wtile(wqkv_eff[:, :2048], 16, 128)       # [16,128,CT,128]
    wqk_t = np.empty((8, 128, 2, CT, 128), fp8np)
    for f in range(8):
        wqk_t[f, :, 0] = wqk16[f]
        wqk_t[f, :, 1] = wqk16[8 + f]
    wv_t = wtile(wqkv_eff[:, 2048:], 4, 256)
    bqk_t = np.ascontiguousarray(bias_qkv[:2048].reshape(16, 128).T).astype(f32)
    vbias_t = (OS * bias_qkv[2048:]).astype(f32)

    wp_t = wtile(proj_w, CT, 128)
    wxp_t = np.zeros((1, CT, 2, 128), fp8np)
    wxp_t[0, :, 0, :] = (WS * proj_b).reshape(CT, 128).astype(fp8np)

    w1_eff = n2_g[:, None] * fc1_w
    bias_fc1 = n2_b @ fc1_w + fc1_b
    w1_hl = wtile_hl(w1_eff, NHT, 128)               # [32,128,2,CT,128]
    w1_t = np.ascontiguousarray(
        w1_hl.reshape(NHT // 2, 2, 128, 2, CT, 128)
        .transpose(0, 2, 1, 3, 4, 5))                # [16,128,2,2,CT,128]
    wx1_t = np.zeros((1, NHT, 2, 128), fp8np)
    wx1_t[0, :, 0, :] = (WS * bias_fc1).reshape(NHT, 128).astype(fp8np)

    w2_t = np.ascontiguousarray(
        wtile_hl(fc2_w, CT, 128)[:, :, :W2P])        # [CT,128,W2P,NHT,128]
    wx2_t = np.zeros((1, CT, 2, 128), fp8np)
    wx2_t[0, :, 0, :] = (WS * fc2_b).reshape(CT, 128).astype(fp8np)

    with_bias = bool(np.any(wxp_t) or np.any(wx1_t) or np.any(wx2_t))
    shared = dict(
        wqk=wqk_t, wv=wv_t, wp=wp_t, w1=w1_t, w2=w2_t,
        bqk=bqk_t, vbias=vbias_t,
        ones=np.ones((128, 1), f32),
    )
    if with_bias:
        shared.update(wxp=wxp_t, wx1=wx1_t, wx2=wx2_t)
    in_maps = []
    for c in range(NCORES):
        s = slice(c * RPC, (c + 1) * RPC)
        in_maps.append(dict(
            shared, xT=xT[s], kcD=kcD[s],
            vcC=vcC[s].reshape(RPC, 128, H * KTC * VP)))
    return in_maps, with_bias


def kernel(**inputs) -> np.ndarray:
    in_maps, with_bias = _prepare_in_maps(**inputs)
    runner = _get_runner(with_bias)
    prepared = runner.prepare(in_maps)
    out = runner.run(prepared)
    res = runner.results(out)
    full = np.empty((B, NP, C), np.float32)
    for c in range(NCORES):
        for r in range(RPC):
            full[c * RPC + r] = res[c]["outT"][r].transpose(2, 1, 0).reshape(T, C)
    return full
